# revision 49
# baseline (speedup 1.0000x reference)
"""Bass/Tile kernel for one batch element of the BiMamba encoder layer.

Per core (one batch element):
  - mamba pipeline in transposed [d, L] space, two L-halves (scan state
    carried across the boundary via `carry`)
  - depthwise conv (kernel 2) is folded into the in_proj matmul: two
    row-scaled weight copies (w0, w1) accumulate into the same PSUM with
    the rhs shifted by one column; SiLU + conv bias applied at PSUM
    evacuation on the Act engine
  - selective scan via DVE tensor_tensor_scan; per (d-block, n) the
    y-contribution h*C accumulates across all 32 n directly in PSUM via
    identity matmuls (k-blocks processed in pairs so two [128,1024] f32
    PSUM accumulators fit alongside the matmul banks)
  - b/g elementwise mults split DVE/GPSIMD for engine balance
  - backward direction = same pipeline on the reversed sequence; the
    gate writes through a reversed AP so no extra un-reverse copy
  - out_proj back to [l, d]; residual + LayerNorm (batched sqrt) + FFN
"""
from contextlib import ExitStack

import concourse.bass as bass
import concourse.mybir as mybir
import concourse.tile as tile
from concourse import bacc
from concourse.masks import make_identity

F32 = mybir.dt.float32
BF16 = mybir.dt.bfloat16
AF = mybir.ActivationFunctionType
OP = mybir.AluOpType

L = 2048
LH = 1024          # half length
D = 512            # d_model == d_inner
N = 32             # d_state
DTR = 32           # dt_rank
FF = 2048
EPS = 1e-5
NBLK = 4           # d blocks of 128
LC = 512           # matmul free chunk (one psum bank)
NLCH = LH // LC    # 2 chunks per half
GSIZE = 4          # n-planes per broadcast group
NGROUP = N // GSIZE
KPAIRS = ((0, 1), (2, 3))
POOL_G16 = 15      # g_t -> Pool for n%16 < POOL_G16, else DVE
POOL_B16 = 0       # b_t -> Pool for n%16 < POOL_B16, else DVE


def emit(ctx: ExitStack, tc: tile.TileContext, io: dict, reps: int = 1):
    for rep in range(reps):
        if rep:
            tc.strict_bb_all_engine_barrier()
        with ExitStack() as rep_ctx:
            _emit_once(rep_ctx, tc, io)


def _emit_once(ctx: ExitStack, tc: tile.TileContext, io: dict):
    nc = tc.nc

    singles = ctx.enter_context(tc.tile_pool(name="singles", bufs=1))
    wpool = ctx.enter_context(tc.tile_pool(name="wpool", bufs=1))
    dirp = ctx.enter_context(tc.tile_pool(name="dirp", bufs=1))
    work = ctx.enter_context(tc.tile_pool(name="work", bufs=2))
    scanp = ctx.enter_context(tc.tile_pool(name="scanp", bufs=2))
    repp = ctx.enter_context(tc.tile_pool(name="repp", bufs=1))
    psmm = ctx.enter_context(tc.tile_pool(name="psmm", bufs=2, space="PSUM"))
    pstr = ctx.enter_context(tc.tile_pool(name="pstr", bufs=2, space="PSUM"))
    psy = ctx.enter_context(tc.tile_pool(name="psy", bufs=1, space="PSUM"))
    dramp = ctx.enter_context(tc.tile_pool(name="dramp", bufs=2, space="DRAM"))

    ident_f = singles.tile([128, 128], F32)
    make_identity(nc, ident_f)
    ident_b = singles.tile([128, 128], BF16)
    nc.vector.tensor_copy(out=ident_b, in_=ident_f)
    eps_col = singles.tile([128, 1], F32)
    nc.vector.memset(eps_col, EPS)

    eidx = [0]

    def _copy(out, in_):
        eidx[0] += 1
        if eidx[0] % 2 == 0:
            nc.scalar.activation(out=out, in_=in_, func=AF.Copy)
        else:
            nc.vector.tensor_copy(out=out, in_=in_)

    # ---------------- weight prep ----------------
    def load_weight_T(dram, rows, cols, name, tags=None, pool=wpool,
                      row_off=0, scale1d=None, dst=None, force_eng=None):
        """dram [row_off:row_off+rows, :cols] f32 -> transposed bf16 tiles:
        dst[ci] is [128, rows] covering cols [ci*128, (ci+1)*128).
        Optional per-row scale (scale1d: [rows]-ish dram column view)."""
        if dst is None:
            dst = []
            for ci, c0 in enumerate(range(0, cols, 128)):
                kw = dict(tag=tags[ci]) if tags else {}
                dst.append(pool.tile([128, rows], BF16, name=f"{name}T{ci}",
                                     **kw))
        for r0 in range(0, rows, 128):
            pr = min(128, rows - r0)
            for cc in range(0, cols, LC):
                wcols = min(LC, cols - cc)
                src = work.tile([128, wcols], F32, tag="wload",
                                name=f"{name}_ld{r0}_{cc}", bufs=2)
                nc.sync.dma_start(
                    out=src[0:pr],
                    in_=dram[row_off + r0:row_off + r0 + pr, cc:cc + wcols])
                tin = src
                idn = ident_f
                if scale1d is not None:
                    scol = work.tile([128, 1], F32, tag="wscol",
                                     name=f"{name}_sc{r0}", bufs=2)
                    nc.sync.dma_start(out=scol[0:pr],
                                      in_=scale1d[row_off + r0:
                                                  row_off + r0 + pr])
                    tin = work.tile([128, wcols], BF16, tag="wsc",
                                    name=f"{name}_scl{r0}_{cc}", bufs=1)
                    nc.vector.tensor_scalar(out=tin[0:pr], in0=src[0:pr],
                                            scalar1=scol[0:pr, 0:1],
                                            scalar2=None, op0=OP.mult)
                    idn = ident_b
                for c0 in range(cc, cc + wcols, 128):
                    pc = min(128, cols - c0)
                    ps = pstr.tile([128, 128], tin.dtype, tag="tr_ps")
                    nc.tensor.transpose(
                        out=ps[0:pc, 0:pr],
                        in_=tin[0:pr, c0 - cc:c0 - cc + pc],
                        identity=idn[0:pr, 0:pr])
                    if force_eng == 'vector':
                        nc.vector.tensor_copy(
                            out=dst[c0 // 128][0:pc, r0:r0 + pr],
                            in_=ps[0:pc, 0:pr])
                    else:
                        _copy(dst[c0 // 128][0:pc, r0:r0 + pr],
                              ps[0:pc, 0:pr])
        return dst

    def load_col(dram_1d, d0, name, pool=wpool):
        t = pool.tile([128, 1], F32, name=name)
        nc.sync.dma_start(out=t, in_=dram_1d[d0:d0 + 128])
        return t

    def prep_dir_early(pfx):
        w = {}
        cw = io[pfx + 'conv_w']
        # in_proj with conv folded: two row-scaled copies of the xs half,
        # plus the unscaled z half.  Tags shared across directions.
        # one DMA per row-chunk of in_w, two conv-scaled transposed copies
        w['iw0T'] = [wpool.tile([128, D], BF16, name=f"{pfx}iw0T{ci}",
                                tag=f"iw0sh{ci}") for ci in range(NBLK)]
        w['iw1T'] = [wpool.tile([128, D], BF16, name=f"{pfx}iw1T{ci}",
                                tag=f"iw1sh{ci}") for ci in range(NBLK)]
        for r0 in range(0, D, 128):
            srcw = work.tile([128, D], F32, tag="wload", bufs=2,
                             name=f"{pfx}iw_ld{r0}")
            nc.sync.dma_start(out=srcw, in_=io[pfx + 'in_w'][r0:r0 + 128, :])
            for vi, dsts in ((0, w['iw0T']), (1, w['iw1T'])):
                scol = work.tile([128, 1], F32, tag=f"wscol{vi}", bufs=2,
                                 name=f"{pfx}iwsc{vi}_{r0}")
                nc.sync.dma_start(out=scol, in_=cw[r0:r0 + 128, vi:vi + 1])
                tin = work.tile([128, D], BF16, tag="wsc", bufs=1,
                                name=f"{pfx}iwscl{vi}_{r0}")
                nc.vector.tensor_scalar(out=tin, in0=srcw, scalar1=scol[:, 0:1],
                                        scalar2=None, op0=OP.mult)
                for c0 in range(0, D, 128):
                    ps = pstr.tile([128, 128], BF16, tag="tr_ps")
                    nc.tensor.transpose(out=ps,
                                        in_=tin[:, c0:c0 + 128],
                                        identity=ident_b)
                    _copy(dsts[c0 // 128][:, r0:r0 + 128], ps)
        w['izT'] = load_weight_T(io[pfx + 'in_w'], D, D, pfx + "iz",
                                 tags=[f"izsh{k}" for k in range(NBLK)],
                                 row_off=D)
        w['cb'] = [load_col(io[pfx + 'conv_b'], k * 128, f"{pfx}cb{k}")
                   for k in range(NBLK)]
        return w

    def prep_dir_late(pfx, w):
        w['owT'] = load_weight_T(io[pfx + 'out_w'], D, D, pfx + "ow")
        w['xpT'] = load_weight_T(io[pfx + 'xproj_w'], 96, D, pfx + "xp",
                                 tags=[f"xpsh{k}" for k in range(NBLK)])
        w['dtwT'] = load_weight_T(io[pfx + 'dt_w'], D, DTR, pfx + "dtw",
                                  tags=["dtwsh"])[0]
        w['A'] = []
        for k in range(NBLK):
            t = work.tile([128, N], F32, tag="aload", name=f"{pfx}Alog{k}")
            nc.sync.dma_start(out=t,
                              in_=io[pfx + 'A_log'][k * 128:(k + 1) * 128, :])
            a = wpool.tile([128, N], F32, name=f"{pfx}A{k}")
            nc.scalar.activation(out=a, in_=t, func=AF.Exp)
            nc.vector.tensor_scalar_mul(out=a, in0=a, scalar1=-1.0)
            w['A'].append(a)
        w['dtb'] = [load_col(io[pfx + 'dt_b'], k * 128, f"{pfx}dtb{k}")
                    for k in range(NBLK)]
        w['Dp'] = [load_col(io[pfx + 'Dp'], k * 128, f"{pfx}Dp{k}")
                   for k in range(NBLK)]
        return w

    def bcast_dram_ap(dram_ap, width):
        """[1, width] dram view -> [128, width] partition-broadcast AP."""
        return bass.AP(tensor=dram_ap.tensor, offset=dram_ap.offset,
                       ap=[[0, 128]] + [list(p) for p in dram_ap.ap[1:]])

    def rep_vec(dram_1d, name):
        tf = work.tile([128, D], F32, name=name + "_repf", tag="xres",
                       bufs=3)
        nc.sync.dma_start(out=tf,
                          in_=bcast_dram_ap(dram_1d[:].unsqueeze(0), D))
        t = singles.tile([128, D], BF16, name=name + "_rep")
        nc.vector.tensor_copy(out=t, in_=tf)
        return t

    g1_rep = rep_vec(io['n1_g'], "g1")
    b1_rep = rep_vec(io['n1_b'], "b1")
    g2_rep = rep_vec(io['n2_g'], "g2")
    b2_rep = rep_vec(io['n2_b'], "b2")
    c2b_rep = rep_vec(io['c2_b'], "c2b")
    c1b = [load_col(io['c1_b'], k * 128, f"c1b{k}") for k in range(FF // 128)]

    # persistent mamba-phase state (y1f split per half so the x1T
    # transposes can later reuse the slots without stalling the pipeline)
    y1fh = [[dirp.tile([128, LH], BF16, tag=f"y1f{h}{k}", name=f"y1f{h}{k}")
             for k in range(NBLK)] for h in (0, 1)]
    x1 = [dirp.tile([128, D], BF16, tag=f"x1_{lt}", name=f"x1_{lt}")
          for lt in range(16)]

    scan_idx = [0]

    # ---------------- mamba stages (emitted interleaved for overlap) ----
    def mamba_p1(w, rev, h, carry, xbound):
        """P1: load + transpose x into xTh (leading boundary column)."""
        tag = f"{'b' if rev else 'f'}{h}"
        st = dict(w=w, rev=rev, h=h, carry=carry, xbound=xbound, tag=tag,
                  xcT=[None] * NBLK, zsil=[None] * NBLK, dtT=[None] * NBLK,
                  dtx=[None] * NBLK, y_sb={})
        lts = range(h * 8, h * 8 + 8) if not rev else \
            range((1 - h) * 8, (1 - h) * 8 + 8)
        xTh = [dirp.tile([128, LH + 1], BF16, tag=f"xTh{k}",
                         name=f"xTh{k}_{tag}") for k in range(NBLK)]
        for k in range(NBLK):
            if h == 0:
                nc.vector.memset(xTh[k][:, 0:1], 0.0)
            else:
                nc.gpsimd.tensor_copy(out=xTh[k][:, 0:1], in_=xbound[k])
        for bi, lt in enumerate(lts):
            xsrc = work.tile([128, D], F32, tag="xres", name=f"x_{tag}_{lt}",
                             bufs=3)
            nc.sync.dma_start(out=xsrc, in_=io['x'][lt * 128:(lt + 1) * 128, :])
            for k in range(NBLK):
                ps = pstr.tile([128, 128], F32, tag="tr_ps")
                nc.tensor.transpose(out=ps,
                                    in_=xsrc[:, k * 128:(k + 1) * 128],
                                    identity=ident_f)
                if not rev:
                    _copy(xTh[k][:, 1 + bi * 128:1 + (bi + 1) * 128], ps)
                else:
                    b = 7 - bi
                    _copy(xTh[k][:, 1 + b * 128:1 + (b + 1) * 128],
                          ps[:, ::-1])
        if h == 0:
            for k in range(NBLK):
                nc.gpsimd.tensor_copy(out=xbound[k],
                                      in_=xTh[k][:, LH:LH + 1])
        st['xTh'] = xTh
        return st

    def mamba_p2(st, ks):
        """P2: in_proj (+folded conv) -> xcT for k in ks (z path separate)."""
        w, tag, xTh, h = st['w'], st['tag'], st['xTh'], st['h']
        for et in ks:
            st['xcT'][et] = dirp.tile([128, LH], BF16, tag=f"xcT{et}",
                                      name=f"xcT{et}_{tag}")
            for lc in range(NLCH):
                ps = psmm.tile([128, LC], F32, tag="mm")
                for k in range(NBLK):
                    nc.tensor.matmul(
                        ps, lhsT=w['iw1T'][k][:, et * 128:(et + 1) * 128],
                        rhs=xTh[k][:, 1 + lc * LC:1 + (lc + 1) * LC],
                        start=(k == 0), stop=False)
                for k in range(NBLK):
                    nc.tensor.matmul(
                        ps, lhsT=w['iw0T'][k][:, et * 128:(et + 1) * 128],
                        rhs=xTh[k][:, lc * LC:(lc + 1) * LC],
                        start=False, stop=(k == NBLK - 1))
                nc.scalar.activation(
                    out=st['xcT'][et][:, lc * LC:(lc + 1) * LC], in_=ps,
                    func=AF.Silu, bias=w['cb'][et], scale=1.0)

    def mamba_p2z(st, ks):
        """z half of in_proj -> zsil (only needed at the P6 gate)."""
        w, tag, xTh = st['w'], st['tag'], st['xTh']
        for et in ks:
            st['zsil'][et] = dirp.tile([128, LH], BF16, tag=f"zsil{et}",
                                       name=f"zsil{et}_{tag}")
            for lc in range(NLCH):
                ps = psmm.tile([128, LC], F32, tag="mm")
                for k in range(NBLK):
                    nc.tensor.matmul(
                        ps, lhsT=w['izT'][k][:, et * 128:(et + 1) * 128],
                        rhs=xTh[k][:, 1 + lc * LC:1 + (lc + 1) * LC],
                        start=(k == 0), stop=(k == NBLK - 1))
                nc.scalar.activation(
                    out=st['zsil'][et][:, lc * LC:(lc + 1) * LC],
                    in_=ps, func=AF.Silu)

    def mamba_p34(st):
        """P3: x_proj -> (dtlin|B|C); P4: dt softplus + dtx."""
        w, tag, xcT = st['w'], st['tag'], st['xcT']
        xp_sb = dirp.tile([96, LH], BF16, tag="xp_sb", name=f"xp_{tag}")
        for lc in range(NLCH):
            ps = psmm.tile([96, LC], F32, tag="mm")
            for k in range(NBLK):
                nc.tensor.matmul(ps, lhsT=w['xpT'][k],
                                 rhs=xcT[k][:, lc * LC:(lc + 1) * LC],
                                 start=(k == 0), stop=(k == NBLK - 1))
            nc.scalar.activation(out=xp_sb[:, lc * LC:(lc + 1) * LC],
                                 in_=ps, func=AF.Copy)
        BCd = dramp.tile([64, LH], BF16, tag="BCd", name=f"BCd_{tag}")
        nc.sync.dma_start(out=BCd, in_=xp_sb[DTR:96, :])
        st['BCd'] = BCd
        spts = {}
        for k in range(NBLK):
            st['dtT'][k] = dirp.tile([128, LH], BF16, tag=f"dtT{k}",
                                     name=f"dtT{k}_{tag}")
            st['dtx'][k] = dirp.tile([128, LH], BF16, tag=f"dtx{k}",
                                     name=f"dtx{k}_{tag}")
            for lc in range(NLCH):
                ps = psmm.tile([128, LC], F32, tag="mm")
                nc.tensor.matmul(
                    ps, lhsT=w['dtwT'][0:DTR, k * 128:(k + 1) * 128],
                    rhs=xp_sb[0:DTR, lc * LC:(lc + 1) * LC],
                    start=True, stop=True)
                # softplus(x) = ln(1 + exp(x))
                spt = work.tile([128, LC], BF16, tag="sptmp", bufs=2,
                                name=f"spt_{tag}_{k}_{lc}")
                nc.scalar.activation(out=spt, in_=ps, func=AF.Exp,
                                     bias=w['dtb'][k], scale=1.0)
                nc.vector.tensor_scalar_add(out=spt, in0=spt, scalar1=1.0)
                nc.scalar.activation(
                    out=st['dtT'][k][:, lc * LC:(lc + 1) * LC],
                    in_=spt, func=AF.Ln)
        for k in range(NBLK):
            nc.vector.tensor_tensor(out=st['dtx'][k], in0=st['dtT'][k],
                                    in1=xcT[k], op=OP.mult)

    def mamba_kp(st, kp):
        """P5 scan pass for one k-pair + P6 drain/gate."""
        w, tag, h, rev = st['w'], st['tag'], st['h'], st['rev']
        carry, BCd = st['carry'], st['BCd']
        dtT, dtx, xcT, zsil = st['dtT'], st['dtx'], st['xcT'], st['zsil']
        ks = KPAIRS[kp]
        psys = {}
        for k in ks:
            psys[k] = psy.tile([128, LH], F32, tag=f"psy{k & 1}",
                               name=f"psy_{tag}_{k}")
        for grp in range(NGROUP):
            reps = []
            for j in range(GSIZE):
                n = grp * GSIZE + j
                br = repp.tile([128, LH], BF16, tag=f"brep{j}",
                               name=f"br{tag}_{kp}_{n}")
                nc.sync.dma_start(out=br,
                                  in_=bcast_dram_ap(BCd[n:n + 1, :], LH))
                cr = repp.tile([128, LH], BF16, tag=f"crep{j}",
                               name=f"cr{tag}_{kp}_{n}")
                nc.sync.dma_start(
                    out=cr, in_=bcast_dram_ap(BCd[N + n:N + n + 1, :], LH))
                reps.append((br, cr))
            for k in ks:
                for j in range(GSIZE):
                    n = grp * GSIZE + j
                    br, cr = reps[j]
                    i = scan_idx[0]
                    scan_idx[0] += 1
                    a_t = scanp.tile([128, LH], BF16, tag="a_t", bufs=3)
                    nc.scalar.activation(out=a_t, in_=dtT[k], func=AF.Exp,
                                         scale=w['A'][k][:, n:n + 1])
                    b_t = scanp.tile([128, LH], BF16, tag="b_t", bufs=3)
                    beng = nc.gpsimd if (i % 16) < POOL_B16 else nc.vector
                    beng.tensor_tensor(out=b_t, in0=dtx[k], in1=br,
                                       op=OP.mult)
                    h_t = scanp.tile([128, LH], BF16, tag="h_t", bufs=3)
                    init = 0.0 if h == 0 else carry[k][:, n:n + 1]
                    nc.vector.tensor_tensor_scan(
                        out=h_t, data0=a_t, data1=b_t, initial=init,
                        op0=OP.mult, op1=OP.add)
                    if h == 0:
                        nc.gpsimd.tensor_copy(out=carry[k][:, n:n + 1],
                                              in_=h_t[:, LH - 1:LH])
                    g_t = scanp.tile([128, LH], BF16, tag="g_t", bufs=2)
                    geng = nc.gpsimd if (i % 16) < POOL_G16 else nc.vector
                    geng.tensor_tensor(out=g_t, in0=h_t, in1=cr, op=OP.mult)
                    first = (grp == 0 and j == 0)
                    last = (grp == NGROUP - 1 and j == GSIZE - 1)
                    for c in range(NLCH):
                        nc.tensor.matmul(
                            psys[k][:, c * LC:(c + 1) * LC], lhsT=ident_b,
                            rhs=g_t[:, c * LC:(c + 1) * LC],
                            start=first, stop=last)
        # P6: drain (fused) + Dp skip + gate
        for k in ks:
            t1 = work.tile([128, LH], BF16, tag="sptmp", bufs=2,
                           name=f"dp_{tag}_{k}")
            nc.vector.tensor_scalar(out=t1, in0=xcT[k], scalar1=w['Dp'][k],
                                    scalar2=None, op0=OP.mult)
            t2 = t1
            nc.vector.tensor_tensor(out=t2, in0=psys[k], in1=t1, op=OP.add)
            if not rev:
                nc.vector.tensor_tensor(
                    out=y1fh[st['h']][k], in0=t2, in1=zsil[k], op=OP.mult)
            else:
                t3 = dirp.tile([128, LH], BF16, tag=f"bst{k}",
                               name=f"y1b{k}_{tag}")
                nc.vector.tensor_tensor(out=t3[:, ::-1], in0=t2,
                                        in1=zsil[k], op=OP.mult)
                st['y_sb'][k] = t3

    # ---------------- out_proj + LN1 (batched sqrt) ----------------
    mvbuf = dirp.tile([128, 16], F32, tag="mvbuf", name="mvbuf")
    rstd8 = dirp.tile([128, 8], F32, tag="rstd8", name="rstd8")

    def make_post(h, wf, wb):
        def post(bstore):
            s1s = {}
            for loc in range(8):
                lt = (1 - h) * 8 + loc
                ps = psmm.tile([128, D], F32, tag="mm")
                for kk in range(NBLK):
                    nc.tensor.matmul(
                        ps,
                        lhsT=y1fh[lt // 8][kk][:, (lt % 8) * 128:
                                               (lt % 8 + 1) * 128],
                        rhs=wf['owT'][kk], start=(kk == 0), stop=False)
                for kk in range(NBLK):
                    nc.tensor.matmul(
                        ps, lhsT=bstore[kk][:, loc * 128:(loc + 1) * 128],
                        rhs=wb['owT'][kk], start=False, stop=(kk == NBLK - 1))
                xres = work.tile([128, D], F32, tag="xres", bufs=3,
                                 name=f"xres{lt}")
                nc.sync.dma_start(out=xres,
                                  in_=io['x'][lt * 128:(lt + 1) * 128, :])
                s1 = dirp.tile([128, D], BF16, tag=f"s1_{loc}",
                               name=f"s1_{lt}")
                nc.vector.tensor_tensor(out=s1, in0=ps, in1=xres, op=OP.add)
                stats = work.tile([128, 6], F32, tag="stats",
                                  name=f"st1_{lt}")
                nc.vector.bn_stats(out=stats, in_=s1)
                nc.vector.bn_aggr(out=mvbuf[:, loc * 2:loc * 2 + 2],
                                  in_=stats)
                s1s[loc] = s1
            nc.scalar.activation(out=rstd8, in_=mvbuf[:, 1:16:2],
                                 func=AF.Sqrt, bias=eps_col[:])
            nc.vector.reciprocal(out=rstd8, in_=rstd8)
            half = 1 - h
            dsts = [dirp.tile([128, LH], BF16, tag=f"y1f{half}{k}",
                              name=f"x1T{half}{k}") for k in range(NBLK)]
            x1T[half] = dsts
            for loc in range(8):
                lt = (1 - h) * 8 + loc
                t = work.tile([128, D], BF16, tag="lnt", bufs=2,
                              name=f"lnt{lt}")
                nc.vector.tensor_scalar(
                    out=t, in0=s1s[loc], scalar1=mvbuf[:, loc * 2:loc * 2 + 1],
                    scalar2=rstd8[:, loc:loc + 1], op0=OP.subtract,
                    op1=OP.mult)
                t2 = work.tile([128, D], BF16, tag="lnt", bufs=2,
                               name=f"lnt2{lt}")
                nc.gpsimd.tensor_tensor(out=t2, in0=t, in1=g1_rep, op=OP.mult)
                nc.gpsimd.tensor_tensor(out=x1[lt], in0=t2, in1=b1_rep,
                                        op=OP.add)
                for k in range(NBLK):
                    ps = pstr.tile([128, 128], BF16, tag="tr_ps")
                    nc.tensor.transpose(out=ps,
                                        in_=x1[lt][:, k * 128:(k + 1) * 128],
                                        identity=ident_b)
                    nc.vector.tensor_copy(
                        out=dsts[k][:, loc * 128:(loc + 1) * 128], in_=ps)
        return post

    # ---------------- x1T + FFN + LN2 ----------------
    # x1T reuses the y1f slots (filled inside the posts)
    x1T = {}

    mv2 = dirp.tile([128, 8], F32, tag="mv2", name="mv2")
    rstd4 = dirp.tile([128, 4], F32, tag="rstd4", name="rstd4")
    FLC = 256

    def ffn_lc(lc, c1T, c2T):
        s2s = {}
        for sub in range(2):
            c = lc * 2 + sub           # global 256-col chunk index
            xt = x1T[c // 4]
            coff = (c % 4) * FLC
            h1 = [work.tile([128, FLC], BF16, tag=f"h1_{ft}", bufs=1,
                            name=f"h1_{ft}_{c}") for ft in range(FF // 128)]
            for ft in range(FF // 128):
                ps = psmm.tile([128, FLC], F32, tag="mm")
                for k in range(NBLK):
                    nc.tensor.matmul(
                        ps, lhsT=c1T[k][:, ft * 128:(ft + 1) * 128],
                        rhs=xt[k][:, coff:coff + FLC],
                        start=(k == 0), stop=(k == NBLK - 1))
                nc.scalar.activation(out=h1[ft], in_=ps, func=AF.Gelu,
                                     bias=c1b[ft], scale=1.0)
            for ls2 in range(2):
                ls = sub * 2 + ls2
                lt = lc * 4 + ls
                ps = psmm.tile([128, D], F32, tag="mm")
                for ft in range(FF // 128):
                    nc.tensor.matmul(
                        ps, lhsT=h1[ft][:, ls2 * 128:(ls2 + 1) * 128],
                        rhs=c2T[ft], start=(ft == 0),
                        stop=(ft == FF // 128 - 1))
                sa = work.tile([128, D], BF16, tag="lnt", bufs=2,
                               name=f"s2a_{lt}")
                nc.vector.tensor_tensor(out=sa, in0=ps, in1=c2b_rep,
                                        op=OP.add)
                s2 = dirp.tile([128, D], BF16, tag=f"s1_{ls + 4}",
                               name=f"s2_{lt}")
                nc.vector.tensor_tensor(out=s2, in0=sa, in1=x1[lt],
                                        op=OP.add)
                stats = work.tile([128, 6], F32, tag="stats",
                                  name=f"st2_{lt}")
                nc.vector.bn_stats(out=stats, in_=s2)
                nc.vector.bn_aggr(out=mv2[:, ls * 2:ls * 2 + 2], in_=stats)
                s2s[ls] = s2
        nc.scalar.activation(out=rstd4, in_=mv2[:, 1:8:2], func=AF.Sqrt,
                             bias=eps_col[:])
        nc.vector.reciprocal(out=rstd4, in_=rstd4)
        for ls in range(4):
            lt = lc * 4 + ls
            t = work.tile([128, D], BF16, tag="lnt", bufs=2, name=f"l2t{lt}")
            nc.vector.tensor_scalar(
                out=t, in0=s2s[ls], scalar1=mv2[:, ls * 2:ls * 2 + 1],
                scalar2=rstd4[:, ls:ls + 1], op0=OP.subtract, op1=OP.mult)
            t2 = work.tile([128, D], BF16, tag="lnt", bufs=2,
                           name=f"l2u{lt}")
            nc.gpsimd.tensor_tensor(out=t2, in0=t, in1=g2_rep, op=OP.mult)
            ot = work.tile([128, D], F32, tag="xres", bufs=3, name=f"ot_{lt}")
            nc.vector.tensor_tensor(out=ot, in0=t2, in1=b2_rep, op=OP.add)
            nc.sync.dma_start(out=io['out'][lt * 128:(lt + 1) * 128, :],
                              in_=ot)

    # ---------------- run (software-pipelined emission) ----------------
    wf = prep_dir_early('f_')
    carry_f = [dirp.tile([128, N], F32, name=f"carryf{k}", tag=f"carryf{k}")
               for k in range(NBLK)]
    xbound_f = [dirp.tile([128, 1], BF16, name=f"xbf{k}", tag=f"xbf{k}")
                for k in range(NBLK)]
    carry_b = [dirp.tile([128, N], F32, name=f"carryb{k}", tag=f"carryb{k}")
               for k in range(NBLK)]
    xbound_b = [dirp.tile([128, 1], BF16, name=f"xbb{k}", tag=f"xbb{k}")
                for k in range(NBLK)]

    stf0 = mamba_p1(wf, False, 0, carry_f, xbound_f)
    mamba_p2(stf0, (0, 1, 2, 3))
    mamba_p2z(stf0, (0, 1, 2, 3))
    prep_dir_late('f_', wf)
    mamba_p34(stf0)
    mamba_kp(stf0, 0)
    stf1 = mamba_p1(wf, False, 1, carry_f, xbound_f)
    mamba_p2(stf1, (0, 1))
    mamba_p2z(stf1, (0, 1))
    mamba_kp(stf0, 1)
    mamba_p2(stf1, (2, 3))
    mamba_p2z(stf1, (2, 3))
    mamba_p34(stf1)
    mamba_kp(stf1, 0)
    wb = prep_dir_early('b_')
    stb0 = mamba_p1(wb, True, 0, carry_b, xbound_b)
    mamba_p2(stb0, (0, 1))
    mamba_p2z(stb0, (0, 1))
    prep_dir_late('b_', wb)
    mamba_kp(stf1, 1)
    mamba_p2(stb0, (2, 3))
    mamba_p2z(stb0, (2, 3))
    mamba_p34(stb0)
    mamba_kp(stb0, 0)
    stb1 = mamba_p1(wb, True, 1, carry_b, xbound_b)
    mamba_p2(stb1, (0, 1))
    mamba_p2z(stb1, (0, 1))
    mamba_kp(stb0, 1)
    make_post(0, wf, wb)(stb0['y_sb'])
    mamba_p2(stb1, (2, 3))
    mamba_p2z(stb1, (2, 3))
    mamba_p34(stb1)
    c1T = load_weight_T(io['c1_w'], FF, D, "c1",
                        tags=[f"c1sh{k}" for k in range(NBLK)],
                        force_eng='vector')
    c2tags = ([f"iw0sh{k}" for k in range(NBLK)]
              + [f"iw1sh{k}" for k in range(NBLK)]
              + [f"izsh{k}" for k in range(NBLK)]
              + [f"c2sh{k}" for k in range(4)])
    c2T = load_weight_T(io['c2_w'], D, FF, "c2", tags=c2tags,
                        force_eng='vector')
    mamba_kp(stb1, 0)
    ffn_lc(2, c1T, c2T)
    ffn_lc(3, c1T, c2T)
    mamba_kp(stb1, 1)
    make_post(1, wf, wb)(stb1['y_sb'])
    ffn_lc(0, c1T, c2T)
    ffn_lc(1, c1T, c2T)


WEIGHT_SPECS = [
    ('in_w', (2 * D, D)), ('conv_w', (D, 2)), ('conv_b', (D,)),
    ('xproj_w', (96, D)), ('dt_w', (D, DTR)), ('dt_b', (D,)),
    ('A_log', (D, N)), ('Dp', (D,)), ('out_w', (D, D)),
]
GLOBAL_SPECS = [
    ('c1_w', (FF, D)), ('c1_b', (FF,)), ('c2_w', (D, FF)), ('c2_b', (D,)),
    ('n1_g', (D,)), ('n1_b', (D,)), ('n2_g', (D,)), ('n2_b', (D,)),
]


def build(debug=False, reps=1):
    nc = bacc.Bacc("TRN2", target_bir_lowering=False, debug=debug)
    io = {}
    io['x'] = nc.declare_dram_parameter('x', [L, D], F32, isOutput=False)
    for pfx in ('f_', 'b_'):
        for name, shape in WEIGHT_SPECS:
            io[pfx + name] = nc.declare_dram_parameter(
                pfx + name, list(shape), F32, isOutput=False)
    for name, shape in GLOBAL_SPECS:
        io[name] = nc.declare_dram_parameter(name, list(shape), F32,
                                             isOutput=False)
    io['out'] = nc.declare_dram_parameter('out', [L, D], F32, isOutput=True)
    with tile.TileContext(nc) as tc:
        with ExitStack() as ctx:
            emit(ctx, tc, io, reps=reps)
    nc.compile()
    return nc


# ======================= SPMD runner =======================
import numpy as np

_NC_CACHE = {}


def _get_nc():
    if 'nc' not in _NC_CACHE:
        _NC_CACHE['nc'] = build()
    return _NC_CACHE['nc']


def kernel(**inputs):
    """Full-tensor BiMamba encoder layer on 8 NeuronCores (batch-parallel)."""
    from concourse.bass_utils import run_bass_kernel_spmd

    nc = _get_nc()
    x = np.ascontiguousarray(np.asarray(inputs['x'], dtype=np.float32))
    B = x.shape[0]
    weights = {}
    for pfx in ('f_', 'b_'):
        for name, _ in WEIGHT_SPECS:
            weights[pfx + name] = np.ascontiguousarray(
                np.asarray(inputs[pfx + name], dtype=np.float32))
    for name, _ in GLOBAL_SPECS:
        weights[name] = np.ascontiguousarray(
            np.asarray(inputs[name], dtype=np.float32))
    in_maps = [dict(weights, x=x[i]) for i in range(B)]
    res = run_bass_kernel_spmd(nc, in_maps, list(range(B)))
    return np.stack([res.results[i]['out'] for i in range(B)]).astype(np.float32)


# revision 50
# speedup vs baseline: 1.0010x; 1.0010x over previous
"""Bass/Tile kernel for one batch element of the BiMamba encoder layer.

Per core (one batch element):
  - mamba pipeline in transposed [d, L] space, two L-halves (scan state
    carried across the boundary via `carry`)
  - depthwise conv (kernel 2) is folded into the in_proj matmul: two
    row-scaled weight copies (w0, w1) accumulate into the same PSUM with
    the rhs shifted by one column; SiLU + conv bias applied at PSUM
    evacuation on the Act engine
  - selective scan via DVE tensor_tensor_scan; per (d-block, n) the
    y-contribution h*C accumulates across all 32 n directly in PSUM via
    identity matmuls (k-blocks processed in pairs so two [128,1024] f32
    PSUM accumulators fit alongside the matmul banks)
  - b/g elementwise mults split DVE/GPSIMD for engine balance
  - backward direction = same pipeline on the reversed sequence; the
    gate writes through a reversed AP so no extra un-reverse copy
  - out_proj back to [l, d]; residual + LayerNorm (batched sqrt) + FFN
"""
from contextlib import ExitStack

import concourse.bass as bass
import concourse.mybir as mybir
import concourse.tile as tile
from concourse import bacc
from concourse.masks import make_identity

F32 = mybir.dt.float32
BF16 = mybir.dt.bfloat16
AF = mybir.ActivationFunctionType
OP = mybir.AluOpType

L = 2048
LH = 1024          # half length
D = 512            # d_model == d_inner
N = 32             # d_state
DTR = 32           # dt_rank
FF = 2048
EPS = 1e-5
NBLK = 4           # d blocks of 128
LC = 512           # matmul free chunk (one psum bank)
NLCH = LH // LC    # 2 chunks per half
GSIZE = 4          # n-planes per broadcast group
NGROUP = N // GSIZE
KPAIRS = ((0, 1), (2, 3))
POOL_G16 = 15      # g_t -> Pool for n%16 < POOL_G16, else DVE
POOL_B16 = 0       # b_t -> Pool for n%16 < POOL_B16, else DVE


def emit(ctx: ExitStack, tc: tile.TileContext, io: dict, reps: int = 1):
    for rep in range(reps):
        if rep:
            tc.strict_bb_all_engine_barrier()
        with ExitStack() as rep_ctx:
            _emit_once(rep_ctx, tc, io)


def _emit_once(ctx: ExitStack, tc: tile.TileContext, io: dict):
    nc = tc.nc

    singles = ctx.enter_context(tc.tile_pool(name="singles", bufs=1))
    wpool = ctx.enter_context(tc.tile_pool(name="wpool", bufs=1))
    dirp = ctx.enter_context(tc.tile_pool(name="dirp", bufs=1))
    work = ctx.enter_context(tc.tile_pool(name="work", bufs=2))
    scanp = ctx.enter_context(tc.tile_pool(name="scanp", bufs=2))
    repp = ctx.enter_context(tc.tile_pool(name="repp", bufs=1))
    psmm = ctx.enter_context(tc.tile_pool(name="psmm", bufs=2, space="PSUM"))
    pstr = ctx.enter_context(tc.tile_pool(name="pstr", bufs=2, space="PSUM"))
    psy = ctx.enter_context(tc.tile_pool(name="psy", bufs=1, space="PSUM"))
    dramp = ctx.enter_context(tc.tile_pool(name="dramp", bufs=2, space="DRAM"))

    ident_f = singles.tile([128, 128], F32)
    make_identity(nc, ident_f)
    ident_b = singles.tile([128, 128], BF16)
    nc.vector.tensor_copy(out=ident_b, in_=ident_f)
    eps_col = singles.tile([128, 1], F32)
    nc.vector.memset(eps_col, EPS)

    eidx = [0]

    def _copy(out, in_):
        eidx[0] += 1
        if eidx[0] % 2 == 0:
            nc.scalar.activation(out=out, in_=in_, func=AF.Copy)
        else:
            nc.vector.tensor_copy(out=out, in_=in_)

    # ---------------- weight prep ----------------
    def load_weight_T(dram, rows, cols, name, tags=None, pool=wpool,
                      row_off=0, scale1d=None, dst=None, force_eng=None):
        """dram [row_off:row_off+rows, :cols] f32 -> transposed bf16 tiles:
        dst[ci] is [128, rows] covering cols [ci*128, (ci+1)*128).
        Optional per-row scale (scale1d: [rows]-ish dram column view)."""
        if dst is None:
            dst = []
            for ci, c0 in enumerate(range(0, cols, 128)):
                kw = dict(tag=tags[ci]) if tags else {}
                dst.append(pool.tile([128, rows], BF16, name=f"{name}T{ci}",
                                     **kw))
        for r0 in range(0, rows, 128):
            pr = min(128, rows - r0)
            for cc in range(0, cols, LC):
                wcols = min(LC, cols - cc)
                src = work.tile([128, wcols], F32, tag="wload",
                                name=f"{name}_ld{r0}_{cc}", bufs=2)
                nc.sync.dma_start(
                    out=src[0:pr],
                    in_=dram[row_off + r0:row_off + r0 + pr, cc:cc + wcols])
                tin = src
                idn = ident_f
                if scale1d is not None:
                    scol = work.tile([128, 1], F32, tag="wscol",
                                     name=f"{name}_sc{r0}", bufs=2)
                    nc.sync.dma_start(out=scol[0:pr],
                                      in_=scale1d[row_off + r0:
                                                  row_off + r0 + pr])
                    tin = work.tile([128, wcols], BF16, tag="wsc",
                                    name=f"{name}_scl{r0}_{cc}", bufs=1)
                    nc.vector.tensor_scalar(out=tin[0:pr], in0=src[0:pr],
                                            scalar1=scol[0:pr, 0:1],
                                            scalar2=None, op0=OP.mult)
                    idn = ident_b
                for c0 in range(cc, cc + wcols, 128):
                    pc = min(128, cols - c0)
                    ps = pstr.tile([128, 128], tin.dtype, tag="tr_ps")
                    nc.tensor.transpose(
                        out=ps[0:pc, 0:pr],
                        in_=tin[0:pr, c0 - cc:c0 - cc + pc],
                        identity=idn[0:pr, 0:pr])
                    if force_eng == 'vector':
                        nc.vector.tensor_copy(
                            out=dst[c0 // 128][0:pc, r0:r0 + pr],
                            in_=ps[0:pc, 0:pr])
                    else:
                        _copy(dst[c0 // 128][0:pc, r0:r0 + pr],
                              ps[0:pc, 0:pr])
        return dst

    def load_col(dram_1d, d0, name, pool=wpool):
        t = pool.tile([128, 1], F32, name=name)
        nc.sync.dma_start(out=t, in_=dram_1d[d0:d0 + 128])
        return t

    def prep_dir_early(pfx):
        w = {}
        cw = io[pfx + 'conv_w']
        # in_proj with conv folded: two row-scaled copies of the xs half,
        # plus the unscaled z half.  Tags shared across directions.
        # one DMA per row-chunk of in_w, two conv-scaled transposed copies
        w['iw0T'] = [wpool.tile([128, D], BF16, name=f"{pfx}iw0T{ci}",
                                tag=f"iw0sh{ci}") for ci in range(NBLK)]
        w['iw1T'] = [wpool.tile([128, D], BF16, name=f"{pfx}iw1T{ci}",
                                tag=f"iw1sh{ci}") for ci in range(NBLK)]
        for r0 in range(0, D, 128):
            srcw = work.tile([128, D], F32, tag="wload", bufs=2,
                             name=f"{pfx}iw_ld{r0}")
            nc.sync.dma_start(out=srcw, in_=io[pfx + 'in_w'][r0:r0 + 128, :])
            for vi, dsts in ((0, w['iw0T']), (1, w['iw1T'])):
                scol = work.tile([128, 1], F32, tag=f"wscol{vi}", bufs=2,
                                 name=f"{pfx}iwsc{vi}_{r0}")
                nc.sync.dma_start(out=scol, in_=cw[r0:r0 + 128, vi:vi + 1])
                tin = work.tile([128, D], BF16, tag="wsc", bufs=1,
                                name=f"{pfx}iwscl{vi}_{r0}")
                nc.vector.tensor_scalar(out=tin, in0=srcw, scalar1=scol[:, 0:1],
                                        scalar2=None, op0=OP.mult)
                for c0 in range(0, D, 128):
                    ps = pstr.tile([128, 128], BF16, tag="tr_ps")
                    nc.tensor.transpose(out=ps,
                                        in_=tin[:, c0:c0 + 128],
                                        identity=ident_b)
                    _copy(dsts[c0 // 128][:, r0:r0 + 128], ps)
        w['izT'] = load_weight_T(io[pfx + 'in_w'], D, D, pfx + "iz",
                                 tags=[f"izsh{k}" for k in range(NBLK)],
                                 row_off=D)
        w['cb'] = [load_col(io[pfx + 'conv_b'], k * 128, f"{pfx}cb{k}")
                   for k in range(NBLK)]
        return w

    def prep_dir_late(pfx, w):
        w['owT'] = load_weight_T(io[pfx + 'out_w'], D, D, pfx + "ow")
        w['xpT'] = load_weight_T(io[pfx + 'xproj_w'], 96, D, pfx + "xp",
                                 tags=[f"xpsh{k}" for k in range(NBLK)])
        w['dtwT'] = load_weight_T(io[pfx + 'dt_w'], D, DTR, pfx + "dtw",
                                  tags=["dtwsh"])[0]
        w['A'] = []
        for k in range(NBLK):
            t = work.tile([128, N], F32, tag="aload", name=f"{pfx}Alog{k}")
            nc.sync.dma_start(out=t,
                              in_=io[pfx + 'A_log'][k * 128:(k + 1) * 128, :])
            a = wpool.tile([128, N], F32, name=f"{pfx}A{k}")
            nc.scalar.activation(out=a, in_=t, func=AF.Exp)
            nc.vector.tensor_scalar_mul(out=a, in0=a, scalar1=-1.0)
            w['A'].append(a)
        w['dtb'] = [load_col(io[pfx + 'dt_b'], k * 128, f"{pfx}dtb{k}")
                    for k in range(NBLK)]
        w['Dp'] = [load_col(io[pfx + 'Dp'], k * 128, f"{pfx}Dp{k}")
                   for k in range(NBLK)]
        return w

    def bcast_dram_ap(dram_ap, width):
        """[1, width] dram view -> [128, width] partition-broadcast AP."""
        return bass.AP(tensor=dram_ap.tensor, offset=dram_ap.offset,
                       ap=[[0, 128]] + [list(p) for p in dram_ap.ap[1:]])

    def rep_vec(dram_1d, name):
        tf = work.tile([128, D], F32, name=name + "_repf", tag="xres",
                       bufs=3)
        nc.sync.dma_start(out=tf,
                          in_=bcast_dram_ap(dram_1d[:].unsqueeze(0), D))
        t = singles.tile([128, D], BF16, name=name + "_rep")
        nc.vector.tensor_copy(out=t, in_=tf)
        return t

    g1_rep = rep_vec(io['n1_g'], "g1")
    b1_rep = rep_vec(io['n1_b'], "b1")
    g2_rep = rep_vec(io['n2_g'], "g2")
    b2_rep = rep_vec(io['n2_b'], "b2")
    c2b_rep = rep_vec(io['c2_b'], "c2b")
    c1b = [load_col(io['c1_b'], k * 128, f"c1b{k}") for k in range(FF // 128)]

    # persistent mamba-phase state (y1f split per half so the x1T
    # transposes can later reuse the slots without stalling the pipeline)
    y1fh = [[dirp.tile([128, LH], BF16, tag=f"y1f{h}{k}", name=f"y1f{h}{k}")
             for k in range(NBLK)] for h in (0, 1)]
    x1 = [dirp.tile([128, D], BF16, tag=f"x1_{lt}", name=f"x1_{lt}")
          for lt in range(16)]

    scan_idx = [0]

    # ---------------- mamba stages (emitted interleaved for overlap) ----
    def mamba_p1(w, rev, h, carry, xbound):
        """P1: load + transpose x into xTh (leading boundary column)."""
        tag = f"{'b' if rev else 'f'}{h}"
        st = dict(w=w, rev=rev, h=h, carry=carry, xbound=xbound, tag=tag,
                  xcT=[None] * NBLK, zsil=[None] * NBLK, dtT=[None] * NBLK,
                  dtx=[None] * NBLK, y_sb={})
        lts = range(h * 8, h * 8 + 8) if not rev else \
            range((1 - h) * 8, (1 - h) * 8 + 8)
        xTh = [dirp.tile([128, LH + 1], BF16, tag=f"xTh{k}",
                         name=f"xTh{k}_{tag}") for k in range(NBLK)]
        for k in range(NBLK):
            if h == 0:
                nc.vector.memset(xTh[k][:, 0:1], 0.0)
            else:
                nc.gpsimd.tensor_copy(out=xTh[k][:, 0:1], in_=xbound[k])
        for bi, lt in enumerate(lts):
            xsrc = work.tile([128, D], F32, tag="xres", name=f"x_{tag}_{lt}",
                             bufs=3)
            nc.sync.dma_start(out=xsrc, in_=io['x'][lt * 128:(lt + 1) * 128, :])
            for k in range(NBLK):
                ps = pstr.tile([128, 128], F32, tag="tr_ps")
                nc.tensor.transpose(out=ps,
                                    in_=xsrc[:, k * 128:(k + 1) * 128],
                                    identity=ident_f)
                if not rev:
                    _copy(xTh[k][:, 1 + bi * 128:1 + (bi + 1) * 128], ps)
                else:
                    b = 7 - bi
                    _copy(xTh[k][:, 1 + b * 128:1 + (b + 1) * 128],
                          ps[:, ::-1])
        if h == 0:
            for k in range(NBLK):
                nc.gpsimd.tensor_copy(out=xbound[k],
                                      in_=xTh[k][:, LH:LH + 1])
        st['xTh'] = xTh
        return st

    def mamba_p2(st, ks):
        """P2: in_proj (+folded conv) -> xcT for k in ks (z path separate)."""
        w, tag, xTh, h = st['w'], st['tag'], st['xTh'], st['h']
        for et in ks:
            st['xcT'][et] = dirp.tile([128, LH], BF16, tag=f"xcT{et}",
                                      name=f"xcT{et}_{tag}")
            for lc in range(NLCH):
                ps = psmm.tile([128, LC], F32, tag="mm")
                for k in range(NBLK):
                    nc.tensor.matmul(
                        ps, lhsT=w['iw1T'][k][:, et * 128:(et + 1) * 128],
                        rhs=xTh[k][:, 1 + lc * LC:1 + (lc + 1) * LC],
                        start=(k == 0), stop=False)
                for k in range(NBLK):
                    nc.tensor.matmul(
                        ps, lhsT=w['iw0T'][k][:, et * 128:(et + 1) * 128],
                        rhs=xTh[k][:, lc * LC:(lc + 1) * LC],
                        start=False, stop=(k == NBLK - 1))
                nc.scalar.activation(
                    out=st['xcT'][et][:, lc * LC:(lc + 1) * LC], in_=ps,
                    func=AF.Silu, bias=w['cb'][et], scale=1.0)

    def mamba_p2z(st, ks):
        """z half of in_proj -> zsil (only needed at the P6 gate)."""
        w, tag, xTh = st['w'], st['tag'], st['xTh']
        for et in ks:
            st['zsil'][et] = dirp.tile([128, LH], BF16, tag=f"zsil{et}",
                                       name=f"zsil{et}_{tag}")
            for lc in range(NLCH):
                ps = psmm.tile([128, LC], F32, tag="mm")
                for k in range(NBLK):
                    nc.tensor.matmul(
                        ps, lhsT=w['izT'][k][:, et * 128:(et + 1) * 128],
                        rhs=xTh[k][:, 1 + lc * LC:1 + (lc + 1) * LC],
                        start=(k == 0), stop=(k == NBLK - 1))
                nc.scalar.activation(
                    out=st['zsil'][et][:, lc * LC:(lc + 1) * LC],
                    in_=ps, func=AF.Silu)

    def mamba_p34(st):
        """P3: x_proj -> (dtlin|B|C); P4: dt softplus + dtx."""
        w, tag, xcT = st['w'], st['tag'], st['xcT']
        xp_sb = dirp.tile([96, LH], BF16, tag="xp_sb", name=f"xp_{tag}")
        for lc in range(NLCH):
            ps = psmm.tile([96, LC], F32, tag="mm")
            for k in range(NBLK):
                nc.tensor.matmul(ps, lhsT=w['xpT'][k],
                                 rhs=xcT[k][:, lc * LC:(lc + 1) * LC],
                                 start=(k == 0), stop=(k == NBLK - 1))
            nc.scalar.activation(out=xp_sb[:, lc * LC:(lc + 1) * LC],
                                 in_=ps, func=AF.Copy)
        BCd = dramp.tile([64, LH], BF16, tag="BCd", name=f"BCd_{tag}")
        nc.sync.dma_start(out=BCd, in_=xp_sb[DTR:96, :])
        st['BCd'] = BCd
        spts = {}
        for k in range(NBLK):
            st['dtT'][k] = dirp.tile([128, LH], BF16, tag=f"dtT{k}",
                                     name=f"dtT{k}_{tag}")
            st['dtx'][k] = dirp.tile([128, LH], BF16, tag=f"dtx{k}",
                                     name=f"dtx{k}_{tag}")
            for lc in range(NLCH):
                ps = psmm.tile([128, LC], F32, tag="mm")
                nc.tensor.matmul(
                    ps, lhsT=w['dtwT'][0:DTR, k * 128:(k + 1) * 128],
                    rhs=xp_sb[0:DTR, lc * LC:(lc + 1) * LC],
                    start=True, stop=True)
                # softplus(x) = ln(1 + exp(x))
                spt = work.tile([128, LC], BF16, tag="sptmp", bufs=2,
                                name=f"spt_{tag}_{k}_{lc}")
                nc.scalar.activation(out=spt, in_=ps, func=AF.Exp,
                                     bias=w['dtb'][k], scale=1.0)
                nc.vector.tensor_scalar_add(out=spt, in0=spt, scalar1=1.0)
                nc.scalar.activation(
                    out=st['dtT'][k][:, lc * LC:(lc + 1) * LC],
                    in_=spt, func=AF.Ln)
        for k in range(NBLK):
            nc.vector.tensor_tensor(out=st['dtx'][k], in0=st['dtT'][k],
                                    in1=xcT[k], op=OP.mult)

    def mamba_kp(st, kp):
        """P5 scan pass for one k-pair + P6 drain/gate."""
        w, tag, h, rev = st['w'], st['tag'], st['h'], st['rev']
        carry, BCd = st['carry'], st['BCd']
        dtT, dtx, xcT, zsil = st['dtT'], st['dtx'], st['xcT'], st['zsil']
        ks = KPAIRS[kp]
        psys = {}
        for k in ks:
            psys[k] = psy.tile([128, LH], F32, tag=f"psy{k & 1}",
                               name=f"psy_{tag}_{k}")
        for grp in range(NGROUP):
            reps = []
            for j in range(GSIZE):
                n = grp * GSIZE + j
                br = repp.tile([128, LH], BF16, tag=f"brep{j}",
                               name=f"br{tag}_{kp}_{n}")
                nc.sync.dma_start(out=br,
                                  in_=bcast_dram_ap(BCd[n:n + 1, :], LH))
                cr = repp.tile([128, LH], BF16, tag=f"crep{j}",
                               name=f"cr{tag}_{kp}_{n}")
                nc.sync.dma_start(
                    out=cr, in_=bcast_dram_ap(BCd[N + n:N + n + 1, :], LH))
                reps.append((br, cr))
            for k in ks:
                for j in range(GSIZE):
                    n = grp * GSIZE + j
                    br, cr = reps[j]
                    i = scan_idx[0]
                    scan_idx[0] += 1
                    a_t = scanp.tile([128, LH], BF16, tag="a_t", bufs=3)
                    nc.scalar.activation(out=a_t, in_=dtT[k], func=AF.Exp,
                                         scale=w['A'][k][:, n:n + 1])
                    b_t = scanp.tile([128, LH], BF16, tag="b_t", bufs=3)
                    beng = nc.gpsimd if (i % 16) < POOL_B16 else nc.vector
                    beng.tensor_tensor(out=b_t, in0=dtx[k], in1=br,
                                       op=OP.mult)
                    h_t = scanp.tile([128, LH], BF16, tag="h_t", bufs=3)
                    init = 0.0 if h == 0 else carry[k][:, n:n + 1]
                    nc.vector.tensor_tensor_scan(
                        out=h_t, data0=a_t, data1=b_t, initial=init,
                        op0=OP.mult, op1=OP.add)
                    if h == 0:
                        nc.gpsimd.tensor_copy(out=carry[k][:, n:n + 1],
                                              in_=h_t[:, LH - 1:LH])
                    g_t = scanp.tile([128, LH], BF16, tag="g_t", bufs=2)
                    geng = nc.gpsimd if (i % 16) < POOL_G16 else nc.vector
                    geng.tensor_tensor(out=g_t, in0=h_t, in1=cr, op=OP.mult)
                    first = (grp == 0 and j == 0)
                    last = (grp == NGROUP - 1 and j == GSIZE - 1)
                    for c in range(NLCH):
                        nc.tensor.matmul(
                            psys[k][:, c * LC:(c + 1) * LC], lhsT=ident_b,
                            rhs=g_t[:, c * LC:(c + 1) * LC],
                            start=first, stop=last)
        # P6: drain (fused) + Dp skip + gate
        for k in ks:
            t1 = work.tile([128, LH], BF16, tag="sptmp", bufs=2,
                           name=f"dp_{tag}_{k}")
            nc.vector.tensor_scalar(out=t1, in0=xcT[k], scalar1=w['Dp'][k],
                                    scalar2=None, op0=OP.mult)
            t2 = t1
            nc.vector.tensor_tensor(out=t2, in0=psys[k], in1=t1, op=OP.add)
            if not rev:
                nc.vector.tensor_tensor(
                    out=y1fh[st['h']][k], in0=t2, in1=zsil[k], op=OP.mult)
            else:
                t3 = dirp.tile([128, LH], BF16, tag=f"bst{k}",
                               name=f"y1b{k}_{tag}")
                nc.vector.tensor_tensor(out=t3[:, ::-1], in0=t2,
                                        in1=zsil[k], op=OP.mult)
                st['y_sb'][k] = t3

    # ---------------- out_proj + LN1 (batched sqrt) ----------------
    mvbuf = dirp.tile([128, 16], F32, tag="mvbuf", name="mvbuf")
    rstd8 = dirp.tile([128, 8], F32, tag="rstd8", name="rstd8")

    def make_post(h, wf, wb):
        def post(bstore):
            s1s = {}
            for loc in range(8):
                lt = (1 - h) * 8 + loc
                ps = psmm.tile([128, D], F32, tag="mm")
                for kk in range(NBLK):
                    nc.tensor.matmul(
                        ps,
                        lhsT=y1fh[lt // 8][kk][:, (lt % 8) * 128:
                                               (lt % 8 + 1) * 128],
                        rhs=wf['owT'][kk], start=(kk == 0), stop=False)
                for kk in range(NBLK):
                    nc.tensor.matmul(
                        ps, lhsT=bstore[kk][:, loc * 128:(loc + 1) * 128],
                        rhs=wb['owT'][kk], start=False, stop=(kk == NBLK - 1))
                xres = work.tile([128, D], F32, tag="xres", bufs=3,
                                 name=f"xres{lt}")
                nc.sync.dma_start(out=xres,
                                  in_=io['x'][lt * 128:(lt + 1) * 128, :])
                s1 = dirp.tile([128, D], BF16, tag=f"s1_{loc}",
                               name=f"s1_{lt}")
                nc.vector.tensor_tensor(out=s1, in0=ps, in1=xres, op=OP.add)
                stats = work.tile([128, 6], F32, tag="stats",
                                  name=f"st1_{lt}")
                nc.vector.bn_stats(out=stats, in_=s1)
                nc.vector.bn_aggr(out=mvbuf[:, loc * 2:loc * 2 + 2],
                                  in_=stats)
                s1s[loc] = s1
            nc.scalar.activation(out=rstd8, in_=mvbuf[:, 1:16:2],
                                 func=AF.Sqrt, bias=eps_col[:])
            nc.vector.reciprocal(out=rstd8, in_=rstd8)
            half = 1 - h
            dsts = [dirp.tile([128, LH], BF16, tag=f"y1f{half}{k}",
                              name=f"x1T{half}{k}") for k in range(NBLK)]
            x1T[half] = dsts
            for loc in range(8):
                lt = (1 - h) * 8 + loc
                t = work.tile([128, D], BF16, tag="lnt", bufs=3,
                              name=f"lnt{lt}")
                nc.vector.tensor_scalar(
                    out=t, in0=s1s[loc], scalar1=mvbuf[:, loc * 2:loc * 2 + 1],
                    scalar2=rstd8[:, loc:loc + 1], op0=OP.subtract,
                    op1=OP.mult)
                t2 = work.tile([128, D], BF16, tag="lnt", bufs=3,
                               name=f"lnt2{lt}")
                nc.gpsimd.tensor_tensor(out=t2, in0=t, in1=g1_rep, op=OP.mult)
                nc.gpsimd.tensor_tensor(out=x1[lt], in0=t2, in1=b1_rep,
                                        op=OP.add)
                for k in range(NBLK):
                    ps = pstr.tile([128, 128], BF16, tag="tr_ps")
                    nc.tensor.transpose(out=ps,
                                        in_=x1[lt][:, k * 128:(k + 1) * 128],
                                        identity=ident_b)
                    nc.vector.tensor_copy(
                        out=dsts[k][:, loc * 128:(loc + 1) * 128], in_=ps)
        return post

    # ---------------- x1T + FFN + LN2 ----------------
    # x1T reuses the y1f slots (filled inside the posts)
    x1T = {}

    mv2 = dirp.tile([128, 8], F32, tag="mv2", name="mv2")
    rstd4 = dirp.tile([128, 4], F32, tag="rstd4", name="rstd4")
    FLC = 256

    def ffn_lc(lc, c1T, c2T):
        s2s = {}
        for sub in range(2):
            c = lc * 2 + sub           # global 256-col chunk index
            xt = x1T[c // 4]
            coff = (c % 4) * FLC
            h1 = [work.tile([128, FLC], BF16, tag=f"h1_{ft}", bufs=1,
                            name=f"h1_{ft}_{c}") for ft in range(FF // 128)]
            for ft in range(FF // 128):
                ps = psmm.tile([128, FLC], F32, tag="mm")
                for k in range(NBLK):
                    nc.tensor.matmul(
                        ps, lhsT=c1T[k][:, ft * 128:(ft + 1) * 128],
                        rhs=xt[k][:, coff:coff + FLC],
                        start=(k == 0), stop=(k == NBLK - 1))
                nc.scalar.activation(out=h1[ft], in_=ps, func=AF.Gelu,
                                     bias=c1b[ft], scale=1.0)
            for ls2 in range(2):
                ls = sub * 2 + ls2
                lt = lc * 4 + ls
                ps = psmm.tile([128, D], F32, tag="mm")
                for ft in range(FF // 128):
                    nc.tensor.matmul(
                        ps, lhsT=h1[ft][:, ls2 * 128:(ls2 + 1) * 128],
                        rhs=c2T[ft], start=(ft == 0),
                        stop=(ft == FF // 128 - 1))
                sa = work.tile([128, D], BF16, tag="lnt", bufs=3,
                               name=f"s2a_{lt}")
                nc.vector.tensor_tensor(out=sa, in0=ps, in1=c2b_rep,
                                        op=OP.add)
                s2 = dirp.tile([128, D], BF16, tag=f"s1_{ls + 4}",
                               name=f"s2_{lt}")
                nc.vector.tensor_tensor(out=s2, in0=sa, in1=x1[lt],
                                        op=OP.add)
                stats = work.tile([128, 6], F32, tag="stats",
                                  name=f"st2_{lt}")
                nc.vector.bn_stats(out=stats, in_=s2)
                nc.vector.bn_aggr(out=mv2[:, ls * 2:ls * 2 + 2], in_=stats)
                s2s[ls] = s2
        nc.scalar.activation(out=rstd4, in_=mv2[:, 1:8:2], func=AF.Sqrt,
                             bias=eps_col[:])
        nc.vector.reciprocal(out=rstd4, in_=rstd4)
        for ls in range(4):
            lt = lc * 4 + ls
            t = work.tile([128, D], BF16, tag="lnt", bufs=3, name=f"l2t{lt}")
            nc.vector.tensor_scalar(
                out=t, in0=s2s[ls], scalar1=mv2[:, ls * 2:ls * 2 + 1],
                scalar2=rstd4[:, ls:ls + 1], op0=OP.subtract, op1=OP.mult)
            t2 = work.tile([128, D], BF16, tag="lnt", bufs=3,
                           name=f"l2u{lt}")
            nc.gpsimd.tensor_tensor(out=t2, in0=t, in1=g2_rep, op=OP.mult)
            ot = work.tile([128, D], F32, tag="xres", bufs=3, name=f"ot_{lt}")
            nc.vector.tensor_tensor(out=ot, in0=t2, in1=b2_rep, op=OP.add)
            nc.sync.dma_start(out=io['out'][lt * 128:(lt + 1) * 128, :],
                              in_=ot)

    # ---------------- run (software-pipelined emission) ----------------
    wf = prep_dir_early('f_')
    carry_f = [dirp.tile([128, N], F32, name=f"carryf{k}", tag=f"carryf{k}")
               for k in range(NBLK)]
    xbound_f = [dirp.tile([128, 1], BF16, name=f"xbf{k}", tag=f"xbf{k}")
                for k in range(NBLK)]
    carry_b = [dirp.tile([128, N], F32, name=f"carryb{k}", tag=f"carryb{k}")
               for k in range(NBLK)]
    xbound_b = [dirp.tile([128, 1], BF16, name=f"xbb{k}", tag=f"xbb{k}")
                for k in range(NBLK)]

    stf0 = mamba_p1(wf, False, 0, carry_f, xbound_f)
    mamba_p2(stf0, (0, 1, 2, 3))
    mamba_p2z(stf0, (0, 1, 2, 3))
    prep_dir_late('f_', wf)
    mamba_p34(stf0)
    mamba_kp(stf0, 0)
    stf1 = mamba_p1(wf, False, 1, carry_f, xbound_f)
    mamba_p2(stf1, (0, 1))
    mamba_p2z(stf1, (0, 1))
    mamba_kp(stf0, 1)
    mamba_p2(stf1, (2, 3))
    mamba_p2z(stf1, (2, 3))
    mamba_p34(stf1)
    mamba_kp(stf1, 0)
    wb = prep_dir_early('b_')
    stb0 = mamba_p1(wb, True, 0, carry_b, xbound_b)
    mamba_p2(stb0, (0, 1))
    mamba_p2z(stb0, (0, 1))
    prep_dir_late('b_', wb)
    mamba_kp(stf1, 1)
    mamba_p2(stb0, (2, 3))
    mamba_p2z(stb0, (2, 3))
    mamba_p34(stb0)
    mamba_kp(stb0, 0)
    stb1 = mamba_p1(wb, True, 1, carry_b, xbound_b)
    mamba_p2(stb1, (0, 1))
    mamba_p2z(stb1, (0, 1))
    mamba_kp(stb0, 1)
    make_post(0, wf, wb)(stb0['y_sb'])
    mamba_p2(stb1, (2, 3))
    mamba_p2z(stb1, (2, 3))
    mamba_p34(stb1)
    c1T = load_weight_T(io['c1_w'], FF, D, "c1",
                        tags=[f"c1sh{k}" for k in range(NBLK)],
                        force_eng='vector')
    c2tags = ([f"iw0sh{k}" for k in range(NBLK)]
              + [f"iw1sh{k}" for k in range(NBLK)]
              + [f"izsh{k}" for k in range(NBLK)]
              + [f"c2sh{k}" for k in range(4)])
    c2T = load_weight_T(io['c2_w'], D, FF, "c2", tags=c2tags,
                        force_eng='vector')
    mamba_kp(stb1, 0)
    ffn_lc(2, c1T, c2T)
    ffn_lc(3, c1T, c2T)
    mamba_kp(stb1, 1)
    make_post(1, wf, wb)(stb1['y_sb'])
    ffn_lc(0, c1T, c2T)
    ffn_lc(1, c1T, c2T)


WEIGHT_SPECS = [
    ('in_w', (2 * D, D)), ('conv_w', (D, 2)), ('conv_b', (D,)),
    ('xproj_w', (96, D)), ('dt_w', (D, DTR)), ('dt_b', (D,)),
    ('A_log', (D, N)), ('Dp', (D,)), ('out_w', (D, D)),
]
GLOBAL_SPECS = [
    ('c1_w', (FF, D)), ('c1_b', (FF,)), ('c2_w', (D, FF)), ('c2_b', (D,)),
    ('n1_g', (D,)), ('n1_b', (D,)), ('n2_g', (D,)), ('n2_b', (D,)),
]


def build(debug=False, reps=1):
    nc = bacc.Bacc("TRN2", target_bir_lowering=False, debug=debug)
    io = {}
    io['x'] = nc.declare_dram_parameter('x', [L, D], F32, isOutput=False)
    for pfx in ('f_', 'b_'):
        for name, shape in WEIGHT_SPECS:
            io[pfx + name] = nc.declare_dram_parameter(
                pfx + name, list(shape), F32, isOutput=False)
    for name, shape in GLOBAL_SPECS:
        io[name] = nc.declare_dram_parameter(name, list(shape), F32,
                                             isOutput=False)
    io['out'] = nc.declare_dram_parameter('out', [L, D], F32, isOutput=True)
    with tile.TileContext(nc) as tc:
        with ExitStack() as ctx:
            emit(ctx, tc, io, reps=reps)
    nc.compile()
    return nc


# ======================= SPMD runner =======================
import numpy as np

_NC_CACHE = {}


def _get_nc():
    if 'nc' not in _NC_CACHE:
        _NC_CACHE['nc'] = build()
    return _NC_CACHE['nc']


def kernel(**inputs):
    """Full-tensor BiMamba encoder layer on 8 NeuronCores (batch-parallel)."""
    from concourse.bass_utils import run_bass_kernel_spmd

    nc = _get_nc()
    x = np.ascontiguousarray(np.asarray(inputs['x'], dtype=np.float32))
    B = x.shape[0]
    weights = {}
    for pfx in ('f_', 'b_'):
        for name, _ in WEIGHT_SPECS:
            weights[pfx + name] = np.ascontiguousarray(
                np.asarray(inputs[pfx + name], dtype=np.float32))
    for name, _ in GLOBAL_SPECS:
        weights[name] = np.ascontiguousarray(
            np.asarray(inputs[name], dtype=np.float32))
    in_maps = [dict(weights, x=x[i]) for i in range(B)]
    res = run_bass_kernel_spmd(nc, in_maps, list(range(B)))
    return np.stack([res.results[i]['out'] for i in range(B)]).astype(np.float32)


# revision 53
# speedup vs baseline: 1.0135x; 1.0125x over previous
"""Bass/Tile kernel for one batch element of the BiMamba encoder layer.

Per core (one batch element):
  - mamba pipeline in transposed [d, L] space, two L-halves (scan state
    carried across the boundary via `carry`)
  - depthwise conv (kernel 2) is folded into the in_proj matmul: two
    row-scaled weight copies (w0, w1) accumulate into the same PSUM with
    the rhs shifted by one column; SiLU + conv bias applied at PSUM
    evacuation on the Act engine
  - selective scan via DVE tensor_tensor_scan; per (d-block, n) the
    y-contribution h*C accumulates across all 32 n directly in PSUM via
    identity matmuls (k-blocks processed in pairs so two [128,1024] f32
    PSUM accumulators fit alongside the matmul banks)
  - b/g elementwise mults split DVE/GPSIMD for engine balance
  - backward direction = same pipeline on the reversed sequence; the
    gate writes through a reversed AP so no extra un-reverse copy
  - out_proj back to [l, d]; residual + LayerNorm (batched sqrt) + FFN
"""
from contextlib import ExitStack

import concourse.bass as bass
import concourse.mybir as mybir
import concourse.tile as tile
from concourse import bacc
from concourse.masks import make_identity

F32 = mybir.dt.float32
BF16 = mybir.dt.bfloat16
AF = mybir.ActivationFunctionType
OP = mybir.AluOpType

L = 2048
LH = 1024          # half length
D = 512            # d_model == d_inner
N = 32             # d_state
DTR = 32           # dt_rank
FF = 2048
EPS = 1e-5
NBLK = 4           # d blocks of 128
LC = 512           # matmul free chunk (one psum bank)
NLCH = LH // LC    # 2 chunks per half
GSIZE = 4          # n-planes per broadcast group
NGROUP = N // GSIZE
KPAIRS = ((0, 1), (2, 3))
POOL_G16 = 15      # g_t -> Pool for n%16 < POOL_G16, else DVE
POOL_B16 = 0       # b_t -> Pool for n%16 < POOL_B16, else DVE


def emit(ctx: ExitStack, tc: tile.TileContext, io: dict, reps: int = 1):
    for rep in range(reps):
        if rep:
            tc.strict_bb_all_engine_barrier()
        with ExitStack() as rep_ctx:
            _emit_once(rep_ctx, tc, io)


def _emit_once(ctx: ExitStack, tc: tile.TileContext, io: dict):
    nc = tc.nc

    singles = ctx.enter_context(tc.tile_pool(name="singles", bufs=1))
    wpool = ctx.enter_context(tc.tile_pool(name="wpool", bufs=1))
    dirp = ctx.enter_context(tc.tile_pool(name="dirp", bufs=1))
    work = ctx.enter_context(tc.tile_pool(name="work", bufs=2))
    scanp = ctx.enter_context(tc.tile_pool(name="scanp", bufs=2))
    repp = ctx.enter_context(tc.tile_pool(name="repp", bufs=1))
    psmm = ctx.enter_context(tc.tile_pool(name="psmm", bufs=2, space="PSUM"))
    pstr = ctx.enter_context(tc.tile_pool(name="pstr", bufs=2, space="PSUM"))
    psy = ctx.enter_context(tc.tile_pool(name="psy", bufs=1, space="PSUM"))
    dramp = ctx.enter_context(tc.tile_pool(name="dramp", bufs=2, space="DRAM"))

    ident_f = singles.tile([128, 128], F32)
    make_identity(nc, ident_f)
    ident_b = singles.tile([128, 128], BF16)
    nc.vector.tensor_copy(out=ident_b, in_=ident_f)
    eps_col = singles.tile([128, 1], F32)
    nc.vector.memset(eps_col, EPS)

    eidx = [0]

    def _copy(out, in_):
        eidx[0] += 1
        if eidx[0] % 2 == 0:
            nc.scalar.activation(out=out, in_=in_, func=AF.Copy)
        else:
            nc.vector.tensor_copy(out=out, in_=in_)

    # ---------------- weight prep ----------------
    def load_weight_T(dram, rows, cols, name, tags=None, pool=wpool,
                      row_off=0, scale1d=None, dst=None, force_eng=None):
        """dram [row_off:row_off+rows, :cols] f32 -> transposed bf16 tiles:
        dst[ci] is [128, rows] covering cols [ci*128, (ci+1)*128).
        Optional per-row scale (scale1d: [rows]-ish dram column view)."""
        if dst is None:
            dst = []
            for ci, c0 in enumerate(range(0, cols, 128)):
                kw = dict(tag=tags[ci]) if tags else {}
                dst.append(pool.tile([128, rows], BF16, name=f"{name}T{ci}",
                                     **kw))
        for r0 in range(0, rows, 128):
            pr = min(128, rows - r0)
            for cc in range(0, cols, LC):
                wcols = min(LC, cols - cc)
                src = work.tile([128, wcols], F32, tag="wload",
                                name=f"{name}_ld{r0}_{cc}", bufs=2)
                nc.sync.dma_start(
                    out=src[0:pr],
                    in_=dram[row_off + r0:row_off + r0 + pr, cc:cc + wcols])
                tin = src
                idn = ident_f
                if scale1d is not None:
                    scol = work.tile([128, 1], F32, tag="wscol",
                                     name=f"{name}_sc{r0}", bufs=2)
                    nc.sync.dma_start(out=scol[0:pr],
                                      in_=scale1d[row_off + r0:
                                                  row_off + r0 + pr])
                    tin = work.tile([128, wcols], BF16, tag="wsc",
                                    name=f"{name}_scl{r0}_{cc}", bufs=1)
                    nc.vector.tensor_scalar(out=tin[0:pr], in0=src[0:pr],
                                            scalar1=scol[0:pr, 0:1],
                                            scalar2=None, op0=OP.mult)
                    idn = ident_b
                for c0 in range(cc, cc + wcols, 128):
                    pc = min(128, cols - c0)
                    ps = pstr.tile([128, 128], tin.dtype, tag="tr_ps")
                    nc.tensor.transpose(
                        out=ps[0:pc, 0:pr],
                        in_=tin[0:pr, c0 - cc:c0 - cc + pc],
                        identity=idn[0:pr, 0:pr])
                    if force_eng == 'vector':
                        nc.vector.tensor_copy(
                            out=dst[c0 // 128][0:pc, r0:r0 + pr],
                            in_=ps[0:pc, 0:pr])
                    else:
                        _copy(dst[c0 // 128][0:pc, r0:r0 + pr],
                              ps[0:pc, 0:pr])
        return dst

    def load_col(dram_1d, d0, name, pool=wpool):
        t = pool.tile([128, 1], F32, name=name)
        nc.sync.dma_start(out=t, in_=dram_1d[d0:d0 + 128])
        return t

    def prep_dir_early(pfx):
        w = {}
        cw = io[pfx + 'conv_w']
        # in_proj with conv folded: two row-scaled copies of the xs half,
        # plus the unscaled z half.  Tags shared across directions.
        # one DMA per row-chunk of in_w, two conv-scaled transposed copies
        w['iw0T'] = [wpool.tile([128, D], BF16, name=f"{pfx}iw0T{ci}",
                                tag=f"iw0sh{ci}") for ci in range(NBLK)]
        w['iw1T'] = [wpool.tile([128, D], BF16, name=f"{pfx}iw1T{ci}",
                                tag=f"iw1sh{ci}") for ci in range(NBLK)]
        for r0 in range(0, D, 128):
            srcw = work.tile([128, D], F32, tag="wload", bufs=2,
                             name=f"{pfx}iw_ld{r0}")
            nc.sync.dma_start(out=srcw, in_=io[pfx + 'in_w'][r0:r0 + 128, :])
            for vi, dsts in ((0, w['iw0T']), (1, w['iw1T'])):
                scol = work.tile([128, 1], F32, tag=f"wscol{vi}", bufs=2,
                                 name=f"{pfx}iwsc{vi}_{r0}")
                nc.sync.dma_start(out=scol, in_=cw[r0:r0 + 128, vi:vi + 1])
                tin = work.tile([128, D], BF16, tag="wsc", bufs=1,
                                name=f"{pfx}iwscl{vi}_{r0}")
                nc.vector.tensor_scalar(out=tin, in0=srcw, scalar1=scol[:, 0:1],
                                        scalar2=None, op0=OP.mult)
                for c0 in range(0, D, 128):
                    ps = pstr.tile([128, 128], BF16, tag="tr_ps")
                    nc.tensor.transpose(out=ps,
                                        in_=tin[:, c0:c0 + 128],
                                        identity=ident_b)
                    _copy(dsts[c0 // 128][:, r0:r0 + 128], ps)
        w['izT'] = load_weight_T(io[pfx + 'in_w'], D, D, pfx + "iz",
                                 tags=[f"izsh{k}" for k in range(NBLK)],
                                 row_off=D)
        w['cb'] = [load_col(io[pfx + 'conv_b'], k * 128, f"{pfx}cb{k}")
                   for k in range(NBLK)]
        return w

    def prep_dir_late(pfx, w):
        w['owT'] = load_weight_T(io[pfx + 'out_w'], D, D, pfx + "ow")
        w['xpT'] = load_weight_T(io[pfx + 'xproj_w'], 96, D, pfx + "xp",
                                 tags=[f"xpsh{k}" for k in range(NBLK)])
        w['dtwT'] = load_weight_T(io[pfx + 'dt_w'], D, DTR, pfx + "dtw",
                                  tags=["dtwsh"])[0]
        w['A'] = []
        for k in range(NBLK):
            t = work.tile([128, N], F32, tag="aload", name=f"{pfx}Alog{k}")
            nc.sync.dma_start(out=t,
                              in_=io[pfx + 'A_log'][k * 128:(k + 1) * 128, :])
            a = wpool.tile([128, N], F32, name=f"{pfx}A{k}")
            nc.scalar.activation(out=a, in_=t, func=AF.Exp)
            nc.vector.tensor_scalar_mul(out=a, in0=a, scalar1=-1.0)
            w['A'].append(a)
        w['dtb'] = [load_col(io[pfx + 'dt_b'], k * 128, f"{pfx}dtb{k}")
                    for k in range(NBLK)]
        w['Dp'] = [load_col(io[pfx + 'Dp'], k * 128, f"{pfx}Dp{k}")
                   for k in range(NBLK)]
        return w

    def bcast_dram_ap(dram_ap, width):
        """[1, width] dram view -> [128, width] partition-broadcast AP."""
        return bass.AP(tensor=dram_ap.tensor, offset=dram_ap.offset,
                       ap=[[0, 128]] + [list(p) for p in dram_ap.ap[1:]])

    def rep_vec(dram_1d, name):
        tf = work.tile([128, D], F32, name=name + "_repf", tag="xres",
                       bufs=3)
        nc.sync.dma_start(out=tf,
                          in_=bcast_dram_ap(dram_1d[:].unsqueeze(0), D))
        t = singles.tile([128, D], BF16, name=name + "_rep")
        nc.vector.tensor_copy(out=t, in_=tf)
        return t

    g1_rep = rep_vec(io['n1_g'], "g1")
    b1_rep = rep_vec(io['n1_b'], "b1")
    g2_rep = rep_vec(io['n2_g'], "g2")
    b2_rep = rep_vec(io['n2_b'], "b2")
    c2b_rep = rep_vec(io['c2_b'], "c2b")
    c1b = [load_col(io['c1_b'], k * 128, f"c1b{k}") for k in range(FF // 128)]

    # persistent mamba-phase state (y1f split per half so the x1T
    # transposes can later reuse the slots without stalling the pipeline)
    y1fh = [[dirp.tile([128, LH], BF16, tag=f"y1f{h}{k}", name=f"y1f{h}{k}")
             for k in range(NBLK)] for h in (0, 1)]
    x1 = [dirp.tile([128, D], BF16, tag=f"x1_{lt}", name=f"x1_{lt}")
          for lt in range(16)]

    scan_idx = [0]

    # ---------------- mamba stages (emitted interleaved for overlap) ----
    def mamba_p1(w, rev, h, carry, xbound):
        """P1: load + transpose x into xTh (leading boundary column)."""
        tag = f"{'b' if rev else 'f'}{h}"
        st = dict(w=w, rev=rev, h=h, carry=carry, xbound=xbound, tag=tag,
                  xcT=[None] * NBLK, zsil=[None] * NBLK, dtT=[None] * NBLK,
                  dtx=[None] * NBLK, y_sb={})
        lts = range(h * 8, h * 8 + 8) if not rev else \
            range((1 - h) * 8, (1 - h) * 8 + 8)
        xTh = [dirp.tile([128, LH + 1], BF16, tag=f"xTh{k}",
                         name=f"xTh{k}_{tag}") for k in range(NBLK)]
        for k in range(NBLK):
            if h == 0:
                nc.vector.memset(xTh[k][:, 0:1], 0.0)
            else:
                nc.gpsimd.tensor_copy(out=xTh[k][:, 0:1], in_=xbound[k])
        for bi, lt in enumerate(lts):
            xsrc = work.tile([128, D], F32, tag="xres", name=f"x_{tag}_{lt}",
                             bufs=3)
            nc.sync.dma_start(out=xsrc, in_=io['x'][lt * 128:(lt + 1) * 128, :])
            for k in range(NBLK):
                ps = pstr.tile([128, 128], F32, tag="tr_ps")
                nc.tensor.transpose(out=ps,
                                    in_=xsrc[:, k * 128:(k + 1) * 128],
                                    identity=ident_f)
                if not rev:
                    _copy(xTh[k][:, 1 + bi * 128:1 + (bi + 1) * 128], ps)
                else:
                    b = 7 - bi
                    _copy(xTh[k][:, 1 + b * 128:1 + (b + 1) * 128],
                          ps[:, ::-1])
        if h == 0:
            for k in range(NBLK):
                nc.gpsimd.tensor_copy(out=xbound[k],
                                      in_=xTh[k][:, LH:LH + 1])
        st['xTh'] = xTh
        return st

    def mamba_p2(st, ks):
        """P2: in_proj (+folded conv) -> xcT for k in ks (z path separate)."""
        w, tag, xTh, h = st['w'], st['tag'], st['xTh'], st['h']
        for et in ks:
            st['xcT'][et] = dirp.tile([128, LH], BF16, tag=f"xcT{et}",
                                      name=f"xcT{et}_{tag}")
            for lc in range(NLCH):
                ps = psmm.tile([128, LC], F32, tag="mm")
                for k in range(NBLK):
                    nc.tensor.matmul(
                        ps, lhsT=w['iw1T'][k][:, et * 128:(et + 1) * 128],
                        rhs=xTh[k][:, 1 + lc * LC:1 + (lc + 1) * LC],
                        start=(k == 0), stop=False)
                for k in range(NBLK):
                    nc.tensor.matmul(
                        ps, lhsT=w['iw0T'][k][:, et * 128:(et + 1) * 128],
                        rhs=xTh[k][:, lc * LC:(lc + 1) * LC],
                        start=False, stop=(k == NBLK - 1))
                nc.scalar.activation(
                    out=st['xcT'][et][:, lc * LC:(lc + 1) * LC], in_=ps,
                    func=AF.Silu, bias=w['cb'][et], scale=1.0)

    def mamba_p2z(st, ks):
        """z half of in_proj -> zsil (only needed at the P6 gate)."""
        w, tag, xTh = st['w'], st['tag'], st['xTh']
        for et in ks:
            st['zsil'][et] = dirp.tile([128, LH], BF16, tag=f"zsil{et}",
                                       name=f"zsil{et}_{tag}")
            for lc in range(NLCH):
                ps = psmm.tile([128, LC], F32, tag="mm")
                for k in range(NBLK):
                    nc.tensor.matmul(
                        ps, lhsT=w['izT'][k][:, et * 128:(et + 1) * 128],
                        rhs=xTh[k][:, 1 + lc * LC:1 + (lc + 1) * LC],
                        start=(k == 0), stop=(k == NBLK - 1))
                nc.scalar.activation(
                    out=st['zsil'][et][:, lc * LC:(lc + 1) * LC],
                    in_=ps, func=AF.Silu)

    def mamba_p34(st):
        """P3: x_proj -> (dtlin|B|C); P4: dt softplus + dtx."""
        w, tag, xcT = st['w'], st['tag'], st['xcT']
        xp_sb = dirp.tile([96, LH], BF16, tag="xp_sb", name=f"xp_{tag}")
        for lc in range(NLCH):
            ps = psmm.tile([96, LC], F32, tag="mm")
            for k in range(NBLK):
                nc.tensor.matmul(ps, lhsT=w['xpT'][k],
                                 rhs=xcT[k][:, lc * LC:(lc + 1) * LC],
                                 start=(k == 0), stop=(k == NBLK - 1))
            nc.scalar.activation(out=xp_sb[:, lc * LC:(lc + 1) * LC],
                                 in_=ps, func=AF.Copy)
        BCd = dramp.tile([64, LH], BF16, tag="BCd", name=f"BCd_{tag}")
        nc.sync.dma_start(out=BCd, in_=xp_sb[DTR:96, :])
        st['BCd'] = BCd
        spts = {}
        for k in range(NBLK):
            st['dtT'][k] = dirp.tile([128, LH], BF16, tag=f"dtT{k}",
                                     name=f"dtT{k}_{tag}")
            st['dtx'][k] = dirp.tile([128, LH], BF16, tag=f"dtx{k}",
                                     name=f"dtx{k}_{tag}")
            for lc in range(NLCH):
                ps = psmm.tile([128, LC], F32, tag="mm")
                nc.tensor.matmul(
                    ps, lhsT=w['dtwT'][0:DTR, k * 128:(k + 1) * 128],
                    rhs=xp_sb[0:DTR, lc * LC:(lc + 1) * LC],
                    start=True, stop=True)
                # softplus(x) = ln(1 + exp(x))
                spt = work.tile([128, LC], BF16, tag="sptmp", bufs=2,
                                name=f"spt_{tag}_{k}_{lc}")
                nc.scalar.activation(out=spt, in_=ps, func=AF.Exp,
                                     bias=w['dtb'][k], scale=1.0)
                nc.vector.tensor_scalar_add(out=spt, in0=spt, scalar1=1.0)
                nc.scalar.activation(
                    out=st['dtT'][k][:, lc * LC:(lc + 1) * LC],
                    in_=spt, func=AF.Ln)
        for k in range(NBLK):
            nc.vector.tensor_tensor(out=st['dtx'][k], in0=st['dtT'][k],
                                    in1=xcT[k], op=OP.mult)

    def mamba_kp(st, kp):
        """P5 scan pass for one k-pair + P6 drain/gate."""
        w, tag, h, rev = st['w'], st['tag'], st['h'], st['rev']
        carry, BCd = st['carry'], st['BCd']
        dtT, dtx, xcT, zsil = st['dtT'], st['dtx'], st['xcT'], st['zsil']
        ks = KPAIRS[kp]
        psys = {}
        for k in ks:
            psys[k] = psy.tile([128, LH], F32, tag=f"psy{k & 1}",
                               name=f"psy_{tag}_{k}")
        for grp in range(NGROUP):
            reps = []
            for j in range(GSIZE):
                n = grp * GSIZE + j
                br = repp.tile([128, LH], BF16, tag=f"brep{j}",
                               name=f"br{tag}_{kp}_{n}")
                nc.sync.dma_start(out=br,
                                  in_=bcast_dram_ap(BCd[n:n + 1, :], LH))
                cr = repp.tile([128, LH], BF16, tag=f"crep{j}",
                               name=f"cr{tag}_{kp}_{n}")
                nc.sync.dma_start(
                    out=cr, in_=bcast_dram_ap(BCd[N + n:N + n + 1, :], LH))
                reps.append((br, cr))
            for k in ks:
                for j in range(GSIZE):
                    n = grp * GSIZE + j
                    br, cr = reps[j]
                    i = scan_idx[0]
                    scan_idx[0] += 1
                    a_t = scanp.tile([128, LH], BF16, tag="a_t", bufs=3)
                    nc.scalar.activation(out=a_t, in_=dtT[k], func=AF.Exp,
                                         scale=w['A'][k][:, n:n + 1])
                    b_t = scanp.tile([128, LH], BF16, tag="b_t", bufs=3)
                    beng = nc.gpsimd if (i % 16) < POOL_B16 else nc.vector
                    beng.tensor_tensor(out=b_t, in0=dtx[k], in1=br,
                                       op=OP.mult)
                    h_t = scanp.tile([128, LH], BF16, tag="h_t", bufs=3)
                    init = 0.0 if h == 0 else carry[k][:, n:n + 1]
                    nc.vector.tensor_tensor_scan(
                        out=h_t, data0=a_t, data1=b_t, initial=init,
                        op0=OP.mult, op1=OP.add)
                    if h == 0:
                        nc.vector.tensor_copy(out=carry[k][:, n:n + 1],
                                              in_=h_t[:, LH - 1:LH])
                    g_t = scanp.tile([128, LH], BF16, tag="g_t", bufs=2)
                    geng = nc.gpsimd if (i % 16) < POOL_G16 else nc.vector
                    geng.tensor_tensor(out=g_t, in0=h_t, in1=cr, op=OP.mult)
                    first = (grp == 0 and j == 0)
                    last = (grp == NGROUP - 1 and j == GSIZE - 1)
                    for c in range(NLCH):
                        nc.tensor.matmul(
                            psys[k][:, c * LC:(c + 1) * LC], lhsT=ident_b,
                            rhs=g_t[:, c * LC:(c + 1) * LC],
                            start=first, stop=last)
        # P6: drain (fused) + Dp skip + gate
        for k in ks:
            t1 = work.tile([128, LH], BF16, tag="sptmp", bufs=2,
                           name=f"dp_{tag}_{k}")
            nc.vector.tensor_scalar(out=t1, in0=xcT[k], scalar1=w['Dp'][k],
                                    scalar2=None, op0=OP.mult)
            t2 = t1
            nc.vector.tensor_tensor(out=t2, in0=psys[k], in1=t1, op=OP.add)
            if not rev:
                nc.vector.tensor_tensor(
                    out=y1fh[st['h']][k], in0=t2, in1=zsil[k], op=OP.mult)
            else:
                t3 = dirp.tile([128, LH], BF16, tag=f"bst{k}",
                               name=f"y1b{k}_{tag}")
                nc.vector.tensor_tensor(out=t3[:, ::-1], in0=t2,
                                        in1=zsil[k], op=OP.mult)
                st['y_sb'][k] = t3

    # ---------------- out_proj + LN1 (batched sqrt) ----------------
    mvbuf = dirp.tile([128, 16], F32, tag="mvbuf", name="mvbuf")
    rstd8 = dirp.tile([128, 8], F32, tag="rstd8", name="rstd8")

    def make_post(h, wf, wb):
        def post(bstore):
            s1s = {}
            for loc in range(8):
                lt = (1 - h) * 8 + loc
                ps = psmm.tile([128, D], F32, tag="mm")
                for kk in range(NBLK):
                    nc.tensor.matmul(
                        ps,
                        lhsT=y1fh[lt // 8][kk][:, (lt % 8) * 128:
                                               (lt % 8 + 1) * 128],
                        rhs=wf['owT'][kk], start=(kk == 0), stop=False)
                for kk in range(NBLK):
                    nc.tensor.matmul(
                        ps, lhsT=bstore[kk][:, loc * 128:(loc + 1) * 128],
                        rhs=wb['owT'][kk], start=False, stop=(kk == NBLK - 1))
                xres = work.tile([128, D], F32, tag="xres", bufs=3,
                                 name=f"xres{lt}")
                nc.sync.dma_start(out=xres,
                                  in_=io['x'][lt * 128:(lt + 1) * 128, :])
                s1 = dirp.tile([128, D], BF16, tag=f"s1_{loc}",
                               name=f"s1_{lt}")
                nc.vector.tensor_tensor(out=s1, in0=ps, in1=xres, op=OP.add)
                stats = work.tile([128, 6], F32, tag="stats",
                                  name=f"st1_{lt}")
                nc.vector.bn_stats(out=stats, in_=s1)
                nc.vector.bn_aggr(out=mvbuf[:, loc * 2:loc * 2 + 2],
                                  in_=stats)
                s1s[loc] = s1
            nc.scalar.activation(out=rstd8, in_=mvbuf[:, 1:16:2],
                                 func=AF.Sqrt, bias=eps_col[:])
            nc.vector.reciprocal(out=rstd8, in_=rstd8)
            half = 1 - h
            dsts = [dirp.tile([128, LH], BF16, tag=f"y1f{half}{k}",
                              name=f"x1T{half}{k}") for k in range(NBLK)]
            x1T[half] = dsts
            for loc in range(8):
                lt = (1 - h) * 8 + loc
                t = work.tile([128, D], BF16, tag="lnt", bufs=3,
                              name=f"lnt{lt}")
                nc.vector.tensor_scalar(
                    out=t, in0=s1s[loc], scalar1=mvbuf[:, loc * 2:loc * 2 + 1],
                    scalar2=rstd8[:, loc:loc + 1], op0=OP.subtract,
                    op1=OP.mult)
                t2 = work.tile([128, D], BF16, tag="lnt", bufs=3,
                               name=f"lnt2{lt}")
                nc.gpsimd.tensor_tensor(out=t2, in0=t, in1=g1_rep, op=OP.mult)
                nc.gpsimd.tensor_tensor(out=x1[lt], in0=t2, in1=b1_rep,
                                        op=OP.add)
                for k in range(NBLK):
                    ps = pstr.tile([128, 128], BF16, tag="tr_ps")
                    nc.tensor.transpose(out=ps,
                                        in_=x1[lt][:, k * 128:(k + 1) * 128],
                                        identity=ident_b)
                    nc.vector.tensor_copy(
                        out=dsts[k][:, loc * 128:(loc + 1) * 128], in_=ps)
        return post

    # ---------------- x1T + FFN + LN2 ----------------
    # x1T reuses the y1f slots (filled inside the posts)
    x1T = {}

    mv2 = dirp.tile([128, 8], F32, tag="mv2", name="mv2")
    rstd4 = dirp.tile([128, 4], F32, tag="rstd4", name="rstd4")
    FLC = 256

    def ffn_lc(lc, c1T, c2T):
        s2s = {}
        for sub in range(2):
            c = lc * 2 + sub           # global 256-col chunk index
            xt = x1T[c // 4]
            coff = (c % 4) * FLC
            h1 = [work.tile([128, FLC], BF16, tag=f"h1_{ft}", bufs=1,
                            name=f"h1_{ft}_{c}") for ft in range(FF // 128)]
            for ft in range(FF // 128):
                ps = psmm.tile([128, FLC], F32, tag="mm")
                for k in range(NBLK):
                    nc.tensor.matmul(
                        ps, lhsT=c1T[k][:, ft * 128:(ft + 1) * 128],
                        rhs=xt[k][:, coff:coff + FLC],
                        start=(k == 0), stop=(k == NBLK - 1))
                nc.scalar.activation(out=h1[ft], in_=ps, func=AF.Gelu,
                                     bias=c1b[ft], scale=1.0)
            for ls2 in range(2):
                ls = sub * 2 + ls2
                lt = lc * 4 + ls
                ps = psmm.tile([128, D], F32, tag="mm")
                for ft in range(FF // 128):
                    nc.tensor.matmul(
                        ps, lhsT=h1[ft][:, ls2 * 128:(ls2 + 1) * 128],
                        rhs=c2T[ft], start=(ft == 0),
                        stop=(ft == FF // 128 - 1))
                sa = work.tile([128, D], BF16, tag="lnt", bufs=3,
                               name=f"s2a_{lt}")
                nc.vector.tensor_tensor(out=sa, in0=ps, in1=c2b_rep,
                                        op=OP.add)
                s2 = dirp.tile([128, D], BF16, tag=f"s1_{ls + 4}",
                               name=f"s2_{lt}")
                nc.vector.tensor_tensor(out=s2, in0=sa, in1=x1[lt],
                                        op=OP.add)
                stats = work.tile([128, 6], F32, tag="stats",
                                  name=f"st2_{lt}")
                nc.vector.bn_stats(out=stats, in_=s2)
                nc.vector.bn_aggr(out=mv2[:, ls * 2:ls * 2 + 2], in_=stats)
                s2s[ls] = s2
        nc.scalar.activation(out=rstd4, in_=mv2[:, 1:8:2], func=AF.Sqrt,
                             bias=eps_col[:])
        nc.vector.reciprocal(out=rstd4, in_=rstd4)
        for ls in range(4):
            lt = lc * 4 + ls
            t = work.tile([128, D], BF16, tag="lnt", bufs=3, name=f"l2t{lt}")
            nc.vector.tensor_scalar(
                out=t, in0=s2s[ls], scalar1=mv2[:, ls * 2:ls * 2 + 1],
                scalar2=rstd4[:, ls:ls + 1], op0=OP.subtract, op1=OP.mult)
            t2 = work.tile([128, D], BF16, tag="lnt", bufs=3,
                           name=f"l2u{lt}")
            nc.gpsimd.tensor_tensor(out=t2, in0=t, in1=g2_rep, op=OP.mult)
            ot = work.tile([128, D], F32, tag="xres", bufs=3, name=f"ot_{lt}")
            nc.vector.tensor_tensor(out=ot, in0=t2, in1=b2_rep, op=OP.add)
            nc.sync.dma_start(out=io['out'][lt * 128:(lt + 1) * 128, :],
                              in_=ot)

    # ---------------- run (software-pipelined emission) ----------------
    wf = prep_dir_early('f_')
    carry_f = [dirp.tile([128, N], F32, name=f"carryf{k}", tag=f"carryf{k}")
               for k in range(NBLK)]
    xbound_f = [dirp.tile([128, 1], BF16, name=f"xbf{k}", tag=f"xbf{k}")
                for k in range(NBLK)]
    carry_b = [dirp.tile([128, N], F32, name=f"carryb{k}", tag=f"carryb{k}")
               for k in range(NBLK)]
    xbound_b = [dirp.tile([128, 1], BF16, name=f"xbb{k}", tag=f"xbb{k}")
                for k in range(NBLK)]

    stf0 = mamba_p1(wf, False, 0, carry_f, xbound_f)
    mamba_p2(stf0, (0, 1, 2, 3))
    mamba_p2z(stf0, (0, 1, 2, 3))
    prep_dir_late('f_', wf)
    mamba_p34(stf0)
    mamba_kp(stf0, 0)
    stf1 = mamba_p1(wf, False, 1, carry_f, xbound_f)
    mamba_p2(stf1, (0, 1))
    mamba_p2z(stf1, (0, 1))
    mamba_kp(stf0, 1)
    mamba_p2(stf1, (2, 3))
    mamba_p2z(stf1, (2, 3))
    mamba_p34(stf1)
    mamba_kp(stf1, 0)
    wb = prep_dir_early('b_')
    stb0 = mamba_p1(wb, True, 0, carry_b, xbound_b)
    mamba_p2(stb0, (0, 1))
    mamba_p2z(stb0, (0, 1))
    prep_dir_late('b_', wb)
    mamba_kp(stf1, 1)
    mamba_p2(stb0, (2, 3))
    mamba_p2z(stb0, (2, 3))
    mamba_p34(stb0)
    mamba_kp(stb0, 0)
    stb1 = mamba_p1(wb, True, 1, carry_b, xbound_b)
    mamba_p2(stb1, (0, 1))
    mamba_p2z(stb1, (0, 1))
    mamba_kp(stb0, 1)
    make_post(0, wf, wb)(stb0['y_sb'])
    mamba_p2(stb1, (2, 3))
    mamba_p2z(stb1, (2, 3))
    mamba_p34(stb1)
    c1T = load_weight_T(io['c1_w'], FF, D, "c1",
                        tags=[f"c1sh{k}" for k in range(NBLK)],
                        force_eng='vector')
    c2tags = ([f"iw0sh{k}" for k in range(NBLK)]
              + [f"iw1sh{k}" for k in range(NBLK)]
              + [f"izsh{k}" for k in range(NBLK)]
              + [f"c2sh{k}" for k in range(4)])
    c2T = load_weight_T(io['c2_w'], D, FF, "c2", tags=c2tags,
                        force_eng='vector')
    mamba_kp(stb1, 0)
    ffn_lc(2, c1T, c2T)
    ffn_lc(3, c1T, c2T)
    mamba_kp(stb1, 1)
    make_post(1, wf, wb)(stb1['y_sb'])
    ffn_lc(0, c1T, c2T)
    ffn_lc(1, c1T, c2T)


WEIGHT_SPECS = [
    ('in_w', (2 * D, D)), ('conv_w', (D, 2)), ('conv_b', (D,)),
    ('xproj_w', (96, D)), ('dt_w', (D, DTR)), ('dt_b', (D,)),
    ('A_log', (D, N)), ('Dp', (D,)), ('out_w', (D, D)),
]
GLOBAL_SPECS = [
    ('c1_w', (FF, D)), ('c1_b', (FF,)), ('c2_w', (D, FF)), ('c2_b', (D,)),
    ('n1_g', (D,)), ('n1_b', (D,)), ('n2_g', (D,)), ('n2_b', (D,)),
]


def build(debug=False, reps=1):
    nc = bacc.Bacc("TRN2", target_bir_lowering=False, debug=debug)
    io = {}
    io['x'] = nc.declare_dram_parameter('x', [L, D], F32, isOutput=False)
    for pfx in ('f_', 'b_'):
        for name, shape in WEIGHT_SPECS:
            io[pfx + name] = nc.declare_dram_parameter(
                pfx + name, list(shape), F32, isOutput=False)
    for name, shape in GLOBAL_SPECS:
        io[name] = nc.declare_dram_parameter(name, list(shape), F32,
                                             isOutput=False)
    io['out'] = nc.declare_dram_parameter('out', [L, D], F32, isOutput=True)
    with tile.TileContext(nc) as tc:
        with ExitStack() as ctx:
            emit(ctx, tc, io, reps=reps)
    nc.compile()
    return nc


# ======================= SPMD runner =======================
import numpy as np

_NC_CACHE = {}


def _get_nc():
    if 'nc' not in _NC_CACHE:
        _NC_CACHE['nc'] = build()
    return _NC_CACHE['nc']


def kernel(**inputs):
    """Full-tensor BiMamba encoder layer on 8 NeuronCores (batch-parallel)."""
    from concourse.bass_utils import run_bass_kernel_spmd

    nc = _get_nc()
    x = np.ascontiguousarray(np.asarray(inputs['x'], dtype=np.float32))
    B = x.shape[0]
    weights = {}
    for pfx in ('f_', 'b_'):
        for name, _ in WEIGHT_SPECS:
            weights[pfx + name] = np.ascontiguousarray(
                np.asarray(inputs[pfx + name], dtype=np.float32))
    for name, _ in GLOBAL_SPECS:
        weights[name] = np.ascontiguousarray(
            np.asarray(inputs[name], dtype=np.float32))
    in_maps = [dict(weights, x=x[i]) for i in range(B)]
    res = run_bass_kernel_spmd(nc, in_maps, list(range(B)))
    return np.stack([res.results[i]['out'] for i in range(B)]).astype(np.float32)


# revision 54
# speedup vs baseline: 1.0167x; 1.0032x over previous
"""Bass/Tile kernel for one batch element of the BiMamba encoder layer.

Per core (one batch element):
  - mamba pipeline in transposed [d, L] space, two L-halves (scan state
    carried across the boundary via `carry`)
  - depthwise conv (kernel 2) is folded into the in_proj matmul: two
    row-scaled weight copies (w0, w1) accumulate into the same PSUM with
    the rhs shifted by one column; SiLU + conv bias applied at PSUM
    evacuation on the Act engine
  - selective scan via DVE tensor_tensor_scan; per (d-block, n) the
    y-contribution h*C accumulates across all 32 n directly in PSUM via
    identity matmuls (k-blocks processed in pairs so two [128,1024] f32
    PSUM accumulators fit alongside the matmul banks)
  - b/g elementwise mults split DVE/GPSIMD for engine balance
  - backward direction = same pipeline on the reversed sequence; the
    gate writes through a reversed AP so no extra un-reverse copy
  - out_proj back to [l, d]; residual + LayerNorm (batched sqrt) + FFN
"""
from contextlib import ExitStack

import concourse.bass as bass
import concourse.mybir as mybir
import concourse.tile as tile
from concourse import bacc
from concourse.masks import make_identity

F32 = mybir.dt.float32
BF16 = mybir.dt.bfloat16
AF = mybir.ActivationFunctionType
OP = mybir.AluOpType

L = 2048
LH = 1024          # half length
D = 512            # d_model == d_inner
N = 32             # d_state
DTR = 32           # dt_rank
FF = 2048
EPS = 1e-5
NBLK = 4           # d blocks of 128
LC = 512           # matmul free chunk (one psum bank)
NLCH = LH // LC    # 2 chunks per half
GSIZE = 4          # n-planes per broadcast group
NGROUP = N // GSIZE
KPAIRS = ((0, 1), (2, 3))
POOL_G16 = 15      # g_t -> Pool for n%16 < POOL_G16, else DVE
POOL_B16 = 0       # b_t -> Pool for n%16 < POOL_B16, else DVE


def emit(ctx: ExitStack, tc: tile.TileContext, io: dict, reps: int = 1):
    for rep in range(reps):
        if rep:
            tc.strict_bb_all_engine_barrier()
        with ExitStack() as rep_ctx:
            _emit_once(rep_ctx, tc, io)


def _emit_once(ctx: ExitStack, tc: tile.TileContext, io: dict):
    nc = tc.nc

    singles = ctx.enter_context(tc.tile_pool(name="singles", bufs=1))
    wpool = ctx.enter_context(tc.tile_pool(name="wpool", bufs=1))
    dirp = ctx.enter_context(tc.tile_pool(name="dirp", bufs=1))
    work = ctx.enter_context(tc.tile_pool(name="work", bufs=2))
    scanp = ctx.enter_context(tc.tile_pool(name="scanp", bufs=2))
    repp = ctx.enter_context(tc.tile_pool(name="repp", bufs=1))
    psmm = ctx.enter_context(tc.tile_pool(name="psmm", bufs=2, space="PSUM"))
    pstr = ctx.enter_context(tc.tile_pool(name="pstr", bufs=2, space="PSUM"))
    psy = ctx.enter_context(tc.tile_pool(name="psy", bufs=1, space="PSUM"))
    dramp = ctx.enter_context(tc.tile_pool(name="dramp", bufs=2, space="DRAM"))

    ident_f = singles.tile([128, 128], F32)
    make_identity(nc, ident_f)
    ident_b = singles.tile([128, 128], BF16)
    nc.vector.tensor_copy(out=ident_b, in_=ident_f)
    eps_col = singles.tile([128, 1], F32)
    nc.vector.memset(eps_col, EPS)

    eidx = [0]

    def _copy(out, in_):
        eidx[0] += 1
        if eidx[0] % 2 == 0:
            nc.scalar.activation(out=out, in_=in_, func=AF.Copy)
        else:
            nc.vector.tensor_copy(out=out, in_=in_)

    # ---------------- weight prep ----------------
    def load_weight_T(dram, rows, cols, name, tags=None, pool=wpool,
                      row_off=0, scale1d=None, dst=None, force_eng=None):
        """dram [row_off:row_off+rows, :cols] f32 -> transposed bf16 tiles:
        dst[ci] is [128, rows] covering cols [ci*128, (ci+1)*128).
        Optional per-row scale (scale1d: [rows]-ish dram column view)."""
        if dst is None:
            dst = []
            for ci, c0 in enumerate(range(0, cols, 128)):
                kw = dict(tag=tags[ci]) if tags else {}
                dst.append(pool.tile([128, rows], BF16, name=f"{name}T{ci}",
                                     **kw))
        for r0 in range(0, rows, 128):
            pr = min(128, rows - r0)
            for cc in range(0, cols, LC):
                wcols = min(LC, cols - cc)
                src = work.tile([128, wcols], F32, tag="wload",
                                name=f"{name}_ld{r0}_{cc}", bufs=2)
                nc.sync.dma_start(
                    out=src[0:pr],
                    in_=dram[row_off + r0:row_off + r0 + pr, cc:cc + wcols])
                tin = src
                idn = ident_f
                if scale1d is not None:
                    scol = work.tile([128, 1], F32, tag="wscol",
                                     name=f"{name}_sc{r0}", bufs=2)
                    nc.sync.dma_start(out=scol[0:pr],
                                      in_=scale1d[row_off + r0:
                                                  row_off + r0 + pr])
                    tin = work.tile([128, wcols], BF16, tag="wsc",
                                    name=f"{name}_scl{r0}_{cc}", bufs=1)
                    nc.vector.tensor_scalar(out=tin[0:pr], in0=src[0:pr],
                                            scalar1=scol[0:pr, 0:1],
                                            scalar2=None, op0=OP.mult)
                    idn = ident_b
                for c0 in range(cc, cc + wcols, 128):
                    pc = min(128, cols - c0)
                    ps = pstr.tile([128, 128], tin.dtype, tag="tr_ps")
                    nc.tensor.transpose(
                        out=ps[0:pc, 0:pr],
                        in_=tin[0:pr, c0 - cc:c0 - cc + pc],
                        identity=idn[0:pr, 0:pr])
                    if force_eng == 'vector':
                        nc.vector.tensor_copy(
                            out=dst[c0 // 128][0:pc, r0:r0 + pr],
                            in_=ps[0:pc, 0:pr])
                    else:
                        _copy(dst[c0 // 128][0:pc, r0:r0 + pr],
                              ps[0:pc, 0:pr])
        return dst

    def load_col(dram_1d, d0, name, pool=wpool):
        t = pool.tile([128, 1], F32, name=name)
        nc.sync.dma_start(out=t, in_=dram_1d[d0:d0 + 128])
        return t

    def prep_dir_early(pfx):
        w = {}
        cw = io[pfx + 'conv_w']
        # in_proj with conv folded: two row-scaled copies of the xs half,
        # plus the unscaled z half.  Tags shared across directions.
        # one DMA per row-chunk of in_w, two conv-scaled transposed copies
        w['iw0T'] = [wpool.tile([128, D], BF16, name=f"{pfx}iw0T{ci}",
                                tag=f"iw0sh{ci}") for ci in range(NBLK)]
        w['iw1T'] = [wpool.tile([128, D], BF16, name=f"{pfx}iw1T{ci}",
                                tag=f"iw1sh{ci}") for ci in range(NBLK)]
        for r0 in range(0, D, 128):
            srcw = work.tile([128, D], F32, tag="wload", bufs=2,
                             name=f"{pfx}iw_ld{r0}")
            nc.sync.dma_start(out=srcw, in_=io[pfx + 'in_w'][r0:r0 + 128, :])
            for vi, dsts in ((0, w['iw0T']), (1, w['iw1T'])):
                scol = work.tile([128, 1], F32, tag=f"wscol{vi}", bufs=2,
                                 name=f"{pfx}iwsc{vi}_{r0}")
                nc.sync.dma_start(out=scol, in_=cw[r0:r0 + 128, vi:vi + 1])
                tin = work.tile([128, D], BF16, tag="wsc", bufs=1,
                                name=f"{pfx}iwscl{vi}_{r0}")
                nc.vector.tensor_scalar(out=tin, in0=srcw, scalar1=scol[:, 0:1],
                                        scalar2=None, op0=OP.mult)
                for c0 in range(0, D, 128):
                    ps = pstr.tile([128, 128], BF16, tag="tr_ps")
                    nc.tensor.transpose(out=ps,
                                        in_=tin[:, c0:c0 + 128],
                                        identity=ident_b)
                    _copy(dsts[c0 // 128][:, r0:r0 + 128], ps)
        w['izT'] = load_weight_T(io[pfx + 'in_w'], D, D, pfx + "iz",
                                 tags=[f"izsh{k}" for k in range(NBLK)],
                                 row_off=D)
        w['cb'] = [load_col(io[pfx + 'conv_b'], k * 128, f"{pfx}cb{k}")
                   for k in range(NBLK)]
        return w

    def prep_dir_late(pfx, w):
        w['owT'] = load_weight_T(io[pfx + 'out_w'], D, D, pfx + "ow")
        w['xpT'] = load_weight_T(io[pfx + 'xproj_w'], 96, D, pfx + "xp",
                                 tags=[f"xpsh{k}" for k in range(NBLK)])
        w['dtwT'] = load_weight_T(io[pfx + 'dt_w'], D, DTR, pfx + "dtw",
                                  tags=["dtwsh"])[0]
        w['A'] = []
        for k in range(NBLK):
            t = work.tile([128, N], F32, tag="aload", name=f"{pfx}Alog{k}")
            nc.sync.dma_start(out=t,
                              in_=io[pfx + 'A_log'][k * 128:(k + 1) * 128, :])
            a = wpool.tile([128, N], F32, name=f"{pfx}A{k}")
            nc.scalar.activation(out=a, in_=t, func=AF.Exp)
            nc.vector.tensor_scalar_mul(out=a, in0=a, scalar1=-1.0)
            w['A'].append(a)
        w['dtb'] = [load_col(io[pfx + 'dt_b'], k * 128, f"{pfx}dtb{k}")
                    for k in range(NBLK)]
        w['Dp'] = [load_col(io[pfx + 'Dp'], k * 128, f"{pfx}Dp{k}")
                   for k in range(NBLK)]
        return w

    def bcast_dram_ap(dram_ap, width):
        """[1, width] dram view -> [128, width] partition-broadcast AP."""
        return bass.AP(tensor=dram_ap.tensor, offset=dram_ap.offset,
                       ap=[[0, 128]] + [list(p) for p in dram_ap.ap[1:]])

    def rep_vec(dram_1d, name):
        tf = work.tile([128, D], F32, name=name + "_repf", tag="xres",
                       bufs=3)
        nc.sync.dma_start(out=tf,
                          in_=bcast_dram_ap(dram_1d[:].unsqueeze(0), D))
        t = singles.tile([128, D], BF16, name=name + "_rep")
        nc.vector.tensor_copy(out=t, in_=tf)
        return t

    g1_rep = rep_vec(io['n1_g'], "g1")
    b1_rep = rep_vec(io['n1_b'], "b1")
    g2_rep = rep_vec(io['n2_g'], "g2")
    b2_rep = rep_vec(io['n2_b'], "b2")
    c2b_rep = rep_vec(io['c2_b'], "c2b")
    c1b = [load_col(io['c1_b'], k * 128, f"c1b{k}") for k in range(FF // 128)]

    # persistent mamba-phase state (y1f split per half so the x1T
    # transposes can later reuse the slots without stalling the pipeline)
    y1fh = [[dirp.tile([128, LH], BF16, tag=f"y1f{h}{k}", name=f"y1f{h}{k}")
             for k in range(NBLK)] for h in (0, 1)]
    x1 = [dirp.tile([128, D], BF16, tag=f"x1_{lt}", name=f"x1_{lt}")
          for lt in range(16)]

    scan_idx = [0]

    # ---------------- mamba stages (emitted interleaved for overlap) ----
    def mamba_p1(w, rev, h, carry, xbound):
        """P1: load + transpose x into xTh (leading boundary column)."""
        tag = f"{'b' if rev else 'f'}{h}"
        st = dict(w=w, rev=rev, h=h, carry=carry, xbound=xbound, tag=tag,
                  xcT=[None] * NBLK, zsil=[None] * NBLK, dtT=[None] * NBLK,
                  dtx=[None] * NBLK, y_sb={})
        lts = range(h * 8, h * 8 + 8) if not rev else \
            range((1 - h) * 8, (1 - h) * 8 + 8)
        xTh = [dirp.tile([128, LH + 1], BF16, tag=f"xTh{k}",
                         name=f"xTh{k}_{tag}") for k in range(NBLK)]
        for k in range(NBLK):
            if h == 0:
                nc.vector.memset(xTh[k][:, 0:1], 0.0)
            else:
                nc.scalar.activation(out=xTh[k][:, 0:1], in_=xbound[k],
                                     func=AF.Copy)
        for bi, lt in enumerate(lts):
            xsrc = work.tile([128, D], F32, tag="xres", name=f"x_{tag}_{lt}",
                             bufs=3)
            nc.sync.dma_start(out=xsrc, in_=io['x'][lt * 128:(lt + 1) * 128, :])
            for k in range(NBLK):
                ps = pstr.tile([128, 128], F32, tag="tr_ps")
                nc.tensor.transpose(out=ps,
                                    in_=xsrc[:, k * 128:(k + 1) * 128],
                                    identity=ident_f)
                if not rev:
                    _copy(xTh[k][:, 1 + bi * 128:1 + (bi + 1) * 128], ps)
                else:
                    b = 7 - bi
                    _copy(xTh[k][:, 1 + b * 128:1 + (b + 1) * 128],
                          ps[:, ::-1])
        if h == 0:
            for k in range(NBLK):
                nc.scalar.activation(out=xbound[k],
                                     in_=xTh[k][:, LH:LH + 1],
                                     func=AF.Copy)
        st['xTh'] = xTh
        return st

    def mamba_p2(st, ks):
        """P2: in_proj (+folded conv) -> xcT for k in ks (z path separate)."""
        w, tag, xTh, h = st['w'], st['tag'], st['xTh'], st['h']
        for et in ks:
            st['xcT'][et] = dirp.tile([128, LH], BF16, tag=f"xcT{et}",
                                      name=f"xcT{et}_{tag}")
            for lc in range(NLCH):
                ps = psmm.tile([128, LC], F32, tag="mm")
                for k in range(NBLK):
                    nc.tensor.matmul(
                        ps, lhsT=w['iw1T'][k][:, et * 128:(et + 1) * 128],
                        rhs=xTh[k][:, 1 + lc * LC:1 + (lc + 1) * LC],
                        start=(k == 0), stop=False)
                for k in range(NBLK):
                    nc.tensor.matmul(
                        ps, lhsT=w['iw0T'][k][:, et * 128:(et + 1) * 128],
                        rhs=xTh[k][:, lc * LC:(lc + 1) * LC],
                        start=False, stop=(k == NBLK - 1))
                nc.scalar.activation(
                    out=st['xcT'][et][:, lc * LC:(lc + 1) * LC], in_=ps,
                    func=AF.Silu, bias=w['cb'][et], scale=1.0)

    def mamba_p2z(st, ks):
        """z half of in_proj -> zsil (only needed at the P6 gate)."""
        w, tag, xTh = st['w'], st['tag'], st['xTh']
        for et in ks:
            st['zsil'][et] = dirp.tile([128, LH], BF16, tag=f"zsil{et}",
                                       name=f"zsil{et}_{tag}")
            for lc in range(NLCH):
                ps = psmm.tile([128, LC], F32, tag="mm")
                for k in range(NBLK):
                    nc.tensor.matmul(
                        ps, lhsT=w['izT'][k][:, et * 128:(et + 1) * 128],
                        rhs=xTh[k][:, 1 + lc * LC:1 + (lc + 1) * LC],
                        start=(k == 0), stop=(k == NBLK - 1))
                nc.scalar.activation(
                    out=st['zsil'][et][:, lc * LC:(lc + 1) * LC],
                    in_=ps, func=AF.Silu)

    def mamba_p34(st):
        """P3: x_proj -> (dtlin|B|C); P4: dt softplus + dtx."""
        w, tag, xcT = st['w'], st['tag'], st['xcT']
        xp_sb = dirp.tile([96, LH], BF16, tag="xp_sb", name=f"xp_{tag}")
        for lc in range(NLCH):
            ps = psmm.tile([96, LC], F32, tag="mm")
            for k in range(NBLK):
                nc.tensor.matmul(ps, lhsT=w['xpT'][k],
                                 rhs=xcT[k][:, lc * LC:(lc + 1) * LC],
                                 start=(k == 0), stop=(k == NBLK - 1))
            nc.scalar.activation(out=xp_sb[:, lc * LC:(lc + 1) * LC],
                                 in_=ps, func=AF.Copy)
        BCd = dramp.tile([64, LH], BF16, tag="BCd", name=f"BCd_{tag}")
        nc.sync.dma_start(out=BCd, in_=xp_sb[DTR:96, :])
        st['BCd'] = BCd
        spts = {}
        for k in range(NBLK):
            st['dtT'][k] = dirp.tile([128, LH], BF16, tag=f"dtT{k}",
                                     name=f"dtT{k}_{tag}")
            st['dtx'][k] = dirp.tile([128, LH], BF16, tag=f"dtx{k}",
                                     name=f"dtx{k}_{tag}")
            for lc in range(NLCH):
                ps = psmm.tile([128, LC], F32, tag="mm")
                nc.tensor.matmul(
                    ps, lhsT=w['dtwT'][0:DTR, k * 128:(k + 1) * 128],
                    rhs=xp_sb[0:DTR, lc * LC:(lc + 1) * LC],
                    start=True, stop=True)
                # softplus(x) = ln(1 + exp(x))
                spt = work.tile([128, LC], BF16, tag="sptmp", bufs=2,
                                name=f"spt_{tag}_{k}_{lc}")
                nc.scalar.activation(out=spt, in_=ps, func=AF.Exp,
                                     bias=w['dtb'][k], scale=1.0)
                nc.vector.tensor_scalar_add(out=spt, in0=spt, scalar1=1.0)
                nc.scalar.activation(
                    out=st['dtT'][k][:, lc * LC:(lc + 1) * LC],
                    in_=spt, func=AF.Ln)
        for k in range(NBLK):
            nc.vector.tensor_tensor(out=st['dtx'][k], in0=st['dtT'][k],
                                    in1=xcT[k], op=OP.mult)

    def mamba_kp(st, kp):
        """P5 scan pass for one k-pair + P6 drain/gate."""
        w, tag, h, rev = st['w'], st['tag'], st['h'], st['rev']
        carry, BCd = st['carry'], st['BCd']
        dtT, dtx, xcT, zsil = st['dtT'], st['dtx'], st['xcT'], st['zsil']
        ks = KPAIRS[kp]
        psys = {}
        for k in ks:
            psys[k] = psy.tile([128, LH], F32, tag=f"psy{k & 1}",
                               name=f"psy_{tag}_{k}")
        for grp in range(NGROUP):
            reps = []
            for j in range(GSIZE):
                n = grp * GSIZE + j
                br = repp.tile([128, LH], BF16, tag=f"brep{j}",
                               name=f"br{tag}_{kp}_{n}")
                nc.sync.dma_start(out=br,
                                  in_=bcast_dram_ap(BCd[n:n + 1, :], LH))
                cr = repp.tile([128, LH], BF16, tag=f"crep{j}",
                               name=f"cr{tag}_{kp}_{n}")
                nc.sync.dma_start(
                    out=cr, in_=bcast_dram_ap(BCd[N + n:N + n + 1, :], LH))
                reps.append((br, cr))
            for k in ks:
                for j in range(GSIZE):
                    n = grp * GSIZE + j
                    br, cr = reps[j]
                    i = scan_idx[0]
                    scan_idx[0] += 1
                    a_t = scanp.tile([128, LH], BF16, tag="a_t", bufs=3)
                    nc.scalar.activation(out=a_t, in_=dtT[k], func=AF.Exp,
                                         scale=w['A'][k][:, n:n + 1])
                    b_t = scanp.tile([128, LH], BF16, tag="b_t", bufs=3)
                    beng = nc.gpsimd if (i % 16) < POOL_B16 else nc.vector
                    beng.tensor_tensor(out=b_t, in0=dtx[k], in1=br,
                                       op=OP.mult)
                    h_t = scanp.tile([128, LH], BF16, tag="h_t", bufs=3)
                    init = 0.0 if h == 0 else carry[k][:, n:n + 1]
                    nc.vector.tensor_tensor_scan(
                        out=h_t, data0=a_t, data1=b_t, initial=init,
                        op0=OP.mult, op1=OP.add)
                    if h == 0:
                        nc.vector.tensor_copy(out=carry[k][:, n:n + 1],
                                              in_=h_t[:, LH - 1:LH])
                    g_t = scanp.tile([128, LH], BF16, tag="g_t", bufs=2)
                    geng = nc.gpsimd if (i % 16) < POOL_G16 else nc.vector
                    geng.tensor_tensor(out=g_t, in0=h_t, in1=cr, op=OP.mult)
                    first = (grp == 0 and j == 0)
                    last = (grp == NGROUP - 1 and j == GSIZE - 1)
                    for c in range(NLCH):
                        nc.tensor.matmul(
                            psys[k][:, c * LC:(c + 1) * LC], lhsT=ident_b,
                            rhs=g_t[:, c * LC:(c + 1) * LC],
                            start=first, stop=last)
        # P6: drain (fused) + Dp skip + gate
        for k in ks:
            t1 = work.tile([128, LH], BF16, tag="sptmp", bufs=2,
                           name=f"dp_{tag}_{k}")
            nc.vector.tensor_scalar(out=t1, in0=xcT[k], scalar1=w['Dp'][k],
                                    scalar2=None, op0=OP.mult)
            t2 = t1
            nc.vector.tensor_tensor(out=t2, in0=psys[k], in1=t1, op=OP.add)
            if not rev:
                nc.vector.tensor_tensor(
                    out=y1fh[st['h']][k], in0=t2, in1=zsil[k], op=OP.mult)
            else:
                t3 = dirp.tile([128, LH], BF16, tag=f"bst{k}",
                               name=f"y1b{k}_{tag}")
                nc.vector.tensor_tensor(out=t3[:, ::-1], in0=t2,
                                        in1=zsil[k], op=OP.mult)
                st['y_sb'][k] = t3

    # ---------------- out_proj + LN1 (batched sqrt) ----------------
    mvbuf = dirp.tile([128, 16], F32, tag="mvbuf", name="mvbuf")
    rstd8 = dirp.tile([128, 8], F32, tag="rstd8", name="rstd8")

    def make_post(h, wf, wb):
        def post(bstore):
            s1s = {}
            for loc in range(8):
                lt = (1 - h) * 8 + loc
                ps = psmm.tile([128, D], F32, tag="mm")
                for kk in range(NBLK):
                    nc.tensor.matmul(
                        ps,
                        lhsT=y1fh[lt // 8][kk][:, (lt % 8) * 128:
                                               (lt % 8 + 1) * 128],
                        rhs=wf['owT'][kk], start=(kk == 0), stop=False)
                for kk in range(NBLK):
                    nc.tensor.matmul(
                        ps, lhsT=bstore[kk][:, loc * 128:(loc + 1) * 128],
                        rhs=wb['owT'][kk], start=False, stop=(kk == NBLK - 1))
                xres = work.tile([128, D], F32, tag="xres", bufs=3,
                                 name=f"xres{lt}")
                nc.sync.dma_start(out=xres,
                                  in_=io['x'][lt * 128:(lt + 1) * 128, :])
                s1 = dirp.tile([128, D], BF16, tag=f"s1_{loc}",
                               name=f"s1_{lt}")
                nc.vector.tensor_tensor(out=s1, in0=ps, in1=xres, op=OP.add)
                stats = work.tile([128, 6], F32, tag="stats",
                                  name=f"st1_{lt}")
                nc.vector.bn_stats(out=stats, in_=s1)
                nc.vector.bn_aggr(out=mvbuf[:, loc * 2:loc * 2 + 2],
                                  in_=stats)
                s1s[loc] = s1
            nc.scalar.activation(out=rstd8, in_=mvbuf[:, 1:16:2],
                                 func=AF.Sqrt, bias=eps_col[:])
            nc.vector.reciprocal(out=rstd8, in_=rstd8)
            half = 1 - h
            dsts = [dirp.tile([128, LH], BF16, tag=f"y1f{half}{k}",
                              name=f"x1T{half}{k}") for k in range(NBLK)]
            x1T[half] = dsts
            for loc in range(8):
                lt = (1 - h) * 8 + loc
                t = work.tile([128, D], BF16, tag="lnt", bufs=3,
                              name=f"lnt{lt}")
                nc.vector.tensor_scalar(
                    out=t, in0=s1s[loc], scalar1=mvbuf[:, loc * 2:loc * 2 + 1],
                    scalar2=rstd8[:, loc:loc + 1], op0=OP.subtract,
                    op1=OP.mult)
                t2 = work.tile([128, D], BF16, tag="lnt", bufs=3,
                               name=f"lnt2{lt}")
                nc.gpsimd.tensor_tensor(out=t2, in0=t, in1=g1_rep, op=OP.mult)
                nc.gpsimd.tensor_tensor(out=x1[lt], in0=t2, in1=b1_rep,
                                        op=OP.add)
                for k in range(NBLK):
                    ps = pstr.tile([128, 128], BF16, tag="tr_ps")
                    nc.tensor.transpose(out=ps,
                                        in_=x1[lt][:, k * 128:(k + 1) * 128],
                                        identity=ident_b)
                    nc.vector.tensor_copy(
                        out=dsts[k][:, loc * 128:(loc + 1) * 128], in_=ps)
        return post

    # ---------------- x1T + FFN + LN2 ----------------
    # x1T reuses the y1f slots (filled inside the posts)
    x1T = {}

    mv2 = dirp.tile([128, 8], F32, tag="mv2", name="mv2")
    rstd4 = dirp.tile([128, 4], F32, tag="rstd4", name="rstd4")
    FLC = 256

    def ffn_lc(lc, c1T, c2T):
        s2s = {}
        for sub in range(2):
            c = lc * 2 + sub           # global 256-col chunk index
            xt = x1T[c // 4]
            coff = (c % 4) * FLC
            h1 = [work.tile([128, FLC], BF16, tag=f"h1_{ft}", bufs=1,
                            name=f"h1_{ft}_{c}") for ft in range(FF // 128)]
            for ft in range(FF // 128):
                ps = psmm.tile([128, FLC], F32, tag="mm")
                for k in range(NBLK):
                    nc.tensor.matmul(
                        ps, lhsT=c1T[k][:, ft * 128:(ft + 1) * 128],
                        rhs=xt[k][:, coff:coff + FLC],
                        start=(k == 0), stop=(k == NBLK - 1))
                nc.scalar.activation(out=h1[ft], in_=ps, func=AF.Gelu,
                                     bias=c1b[ft], scale=1.0)
            for ls2 in range(2):
                ls = sub * 2 + ls2
                lt = lc * 4 + ls
                ps = psmm.tile([128, D], F32, tag="mm")
                for ft in range(FF // 128):
                    nc.tensor.matmul(
                        ps, lhsT=h1[ft][:, ls2 * 128:(ls2 + 1) * 128],
                        rhs=c2T[ft], start=(ft == 0),
                        stop=(ft == FF // 128 - 1))
                sa = work.tile([128, D], BF16, tag="lnt", bufs=3,
                               name=f"s2a_{lt}")
                nc.vector.tensor_tensor(out=sa, in0=ps, in1=c2b_rep,
                                        op=OP.add)
                s2 = dirp.tile([128, D], BF16, tag=f"s1_{ls + 4}",
                               name=f"s2_{lt}")
                nc.vector.tensor_tensor(out=s2, in0=sa, in1=x1[lt],
                                        op=OP.add)
                stats = work.tile([128, 6], F32, tag="stats",
                                  name=f"st2_{lt}")
                nc.vector.bn_stats(out=stats, in_=s2)
                nc.vector.bn_aggr(out=mv2[:, ls * 2:ls * 2 + 2], in_=stats)
                s2s[ls] = s2
        nc.scalar.activation(out=rstd4, in_=mv2[:, 1:8:2], func=AF.Sqrt,
                             bias=eps_col[:])
        nc.vector.reciprocal(out=rstd4, in_=rstd4)
        for ls in range(4):
            lt = lc * 4 + ls
            t = work.tile([128, D], BF16, tag="lnt", bufs=3, name=f"l2t{lt}")
            nc.vector.tensor_scalar(
                out=t, in0=s2s[ls], scalar1=mv2[:, ls * 2:ls * 2 + 1],
                scalar2=rstd4[:, ls:ls + 1], op0=OP.subtract, op1=OP.mult)
            t2 = work.tile([128, D], BF16, tag="lnt", bufs=3,
                           name=f"l2u{lt}")
            geng = nc.vector if lc >= 2 else nc.gpsimd
            geng.tensor_tensor(out=t2, in0=t, in1=g2_rep, op=OP.mult)
            ot = work.tile([128, D], F32, tag="xres", bufs=3, name=f"ot_{lt}")
            nc.vector.tensor_tensor(out=ot, in0=t2, in1=b2_rep, op=OP.add)
            nc.sync.dma_start(out=io['out'][lt * 128:(lt + 1) * 128, :],
                              in_=ot)

    # ---------------- run (software-pipelined emission) ----------------
    wf = prep_dir_early('f_')
    carry_f = [dirp.tile([128, N], F32, name=f"carryf{k}", tag=f"carryf{k}")
               for k in range(NBLK)]
    xbound_f = [dirp.tile([128, 1], BF16, name=f"xbf{k}", tag=f"xbf{k}")
                for k in range(NBLK)]
    carry_b = [dirp.tile([128, N], F32, name=f"carryb{k}", tag=f"carryb{k}")
               for k in range(NBLK)]
    xbound_b = [dirp.tile([128, 1], BF16, name=f"xbb{k}", tag=f"xbb{k}")
                for k in range(NBLK)]

    stf0 = mamba_p1(wf, False, 0, carry_f, xbound_f)
    mamba_p2(stf0, (0, 1, 2, 3))
    mamba_p2z(stf0, (0, 1, 2, 3))
    prep_dir_late('f_', wf)
    mamba_p34(stf0)
    mamba_kp(stf0, 0)
    stf1 = mamba_p1(wf, False, 1, carry_f, xbound_f)
    mamba_p2(stf1, (0, 1))
    mamba_p2z(stf1, (0, 1))
    mamba_kp(stf0, 1)
    mamba_p2(stf1, (2, 3))
    mamba_p2z(stf1, (2, 3))
    mamba_p34(stf1)
    mamba_kp(stf1, 0)
    wb = prep_dir_early('b_')
    stb0 = mamba_p1(wb, True, 0, carry_b, xbound_b)
    mamba_p2(stb0, (0, 1))
    mamba_p2z(stb0, (0, 1))
    prep_dir_late('b_', wb)
    mamba_kp(stf1, 1)
    mamba_p2(stb0, (2, 3))
    mamba_p2z(stb0, (2, 3))
    mamba_p34(stb0)
    mamba_kp(stb0, 0)
    stb1 = mamba_p1(wb, True, 1, carry_b, xbound_b)
    mamba_p2(stb1, (0, 1))
    mamba_p2z(stb1, (0, 1))
    mamba_kp(stb0, 1)
    make_post(0, wf, wb)(stb0['y_sb'])
    mamba_p2(stb1, (2, 3))
    mamba_p2z(stb1, (2, 3))
    mamba_p34(stb1)
    c1T = load_weight_T(io['c1_w'], FF, D, "c1",
                        tags=[f"c1sh{k}" for k in range(NBLK)],
                        force_eng='vector')
    c2tags = ([f"iw0sh{k}" for k in range(NBLK)]
              + [f"iw1sh{k}" for k in range(NBLK)]
              + [f"izsh{k}" for k in range(NBLK)]
              + [f"c2sh{k}" for k in range(4)])
    c2T = load_weight_T(io['c2_w'], D, FF, "c2", tags=c2tags,
                        force_eng='vector')
    mamba_kp(stb1, 0)
    ffn_lc(2, c1T, c2T)
    ffn_lc(3, c1T, c2T)
    mamba_kp(stb1, 1)
    make_post(1, wf, wb)(stb1['y_sb'])
    ffn_lc(0, c1T, c2T)
    ffn_lc(1, c1T, c2T)


WEIGHT_SPECS = [
    ('in_w', (2 * D, D)), ('conv_w', (D, 2)), ('conv_b', (D,)),
    ('xproj_w', (96, D)), ('dt_w', (D, DTR)), ('dt_b', (D,)),
    ('A_log', (D, N)), ('Dp', (D,)), ('out_w', (D, D)),
]
GLOBAL_SPECS = [
    ('c1_w', (FF, D)), ('c1_b', (FF,)), ('c2_w', (D, FF)), ('c2_b', (D,)),
    ('n1_g', (D,)), ('n1_b', (D,)), ('n2_g', (D,)), ('n2_b', (D,)),
]


def build(debug=False, reps=1):
    nc = bacc.Bacc("TRN2", target_bir_lowering=False, debug=debug)
    io = {}
    io['x'] = nc.declare_dram_parameter('x', [L, D], F32, isOutput=False)
    for pfx in ('f_', 'b_'):
        for name, shape in WEIGHT_SPECS:
            io[pfx + name] = nc.declare_dram_parameter(
                pfx + name, list(shape), F32, isOutput=False)
    for name, shape in GLOBAL_SPECS:
        io[name] = nc.declare_dram_parameter(name, list(shape), F32,
                                             isOutput=False)
    io['out'] = nc.declare_dram_parameter('out', [L, D], F32, isOutput=True)
    with tile.TileContext(nc) as tc:
        with ExitStack() as ctx:
            emit(ctx, tc, io, reps=reps)
    nc.compile()
    return nc


# ======================= SPMD runner =======================
import numpy as np

_NC_CACHE = {}


def _get_nc():
    if 'nc' not in _NC_CACHE:
        _NC_CACHE['nc'] = build()
    return _NC_CACHE['nc']


def kernel(**inputs):
    """Full-tensor BiMamba encoder layer on 8 NeuronCores (batch-parallel)."""
    from concourse.bass_utils import run_bass_kernel_spmd

    nc = _get_nc()
    x = np.ascontiguousarray(np.asarray(inputs['x'], dtype=np.float32))
    B = x.shape[0]
    weights = {}
    for pfx in ('f_', 'b_'):
        for name, _ in WEIGHT_SPECS:
            weights[pfx + name] = np.ascontiguousarray(
                np.asarray(inputs[pfx + name], dtype=np.float32))
    for name, _ in GLOBAL_SPECS:
        weights[name] = np.ascontiguousarray(
            np.asarray(inputs[name], dtype=np.float32))
    in_maps = [dict(weights, x=x[i]) for i in range(B)]
    res = run_bass_kernel_spmd(nc, in_maps, list(range(B)))
    return np.stack([res.results[i]['out'] for i in range(B)]).astype(np.float32)
